# revision 1
# baseline (speedup 1.0000x reference)
"""Trainium2 Bass kernel for DTWFeatures.

Problem: x (64,3,1024), patts (32,3,32) -> out (64,32,1024)
  dist[b,p,l,t] = sqrt(max(|x[b,:,t]-patts[p,:,l]|^2, eps))
  DP:  D[l,t] = dist[l,t] + min(D[l-1,t], w*D[l,t-1], w*D[l-1,t-1])
  out[b,p,t] = D[L-1,t]

Strategy (8 cores, data-parallel over batch, 8 batches/core):
  * Rescale E[l,t] = D[l,t]*w^-(t-SHIFT) which removes w from the recurrence:
        E[l,t] = dist'[l,t] + min(E[l,t-1], E[l-1,t], E[l-1,t-1])
    with dist'[l,t] = dist[l,t]*w^-(t-SHIFT).  SHIFT=512 keeps all
    magnitudes within fp32 range (w^-2(t-SHIFT) in [1e-32, 8.7e31]).
  * Per row l this is a first-order recurrence solved by ONE DVE
    tensor_tensor_scan (op0=min, op1=add):
        state_t = min(c_t, state_{t-1}) + dist'_t,  c_t = min(E[l-1,t], E[l-1,t-1])
  * dist'^2 is produced directly by TensorE as a K=17 matmul:
        out[(b4,p), t] = sum_k lhsT[k,(b,p)] * rhs[k,t]
    with lhsT rows = block-diag -2*patts (12), per-b x2-indicators (4),
    p2+eps (1) and rhs rows = x*w2inv (12), x2*w2inv (4), w2inv (1).
    ScalarE (ACT) then applies sqrt PSUM->SBUF.
  * 256 pairs/core = 2 groups of 128 partitions -> two independent
    (window-min -> scan) chains per row that interleave on DVE.  TensorE,
    ScalarE and the DMAs run well ahead; DVE is the bottleneck engine
    (~150us busy; TensorTensor and scan are fp32 1x ops and GPSIMD cannot
    execute TensorTensor at all on trn2 codegen).
"""

import os
import sys

if "/opt/trn_rl_repo" not in sys.path:
    sys.path.insert(0, "/opt/trn_rl_repo")
# the device path runs through jax's axon PJRT backend; make sure a
# harness-pinned JAX_PLATFORMS doesn't hide it (no-op if jax is already up)
if "jax" not in sys.modules and "axon" not in os.environ.get(
    "JAX_PLATFORMS", "axon"
):
    os.environ["JAX_PLATFORMS"] = "axon," + os.environ["JAX_PLATFORMS"]

import numpy as np

NB, ND, NP, NL, NT = 64, 3, 32, 32, 1024   # batch, xdim, n_patts, l_patts, T
NCORES = 8
BPC = NB // NCORES                     # 8 batches per core
RHO = 0.1
W = RHO ** (1.0 / NL)
SHIFT = 512.0
EPS = 2e-5
INF = 1.0e30
K = 17                                 # matmul contraction rows

SEGS = 1         # scan segments per DP row

_CACHE = {}


def _tables():
    if "tables" not in _CACHE:
        t = np.arange(NT, dtype=np.float64)
        w2inv = (W ** (-2.0 * (t - SHIFT))).astype(np.float32)
        wpos = (W ** (t - SHIFT)).astype(np.float32)
        W2INV17 = np.ascontiguousarray(np.tile(w2inv[None, :], (K, 1)))
        # rows 0..11 multiply x in the rhs; carry the |x-p|^2 cross-term's -2
        W2INV17[0:12] *= -2.0
        # unscaled copy for the x^2 rows (partitions 0..7)
        W2INVP8 = np.ascontiguousarray(np.tile(w2inv[None, :], (8, 1)))
        WPOS2 = np.ascontiguousarray(np.tile(wpos[None, None, :], (128, 2, 1)))
        INDIC = np.zeros((4, 128, NL), np.float32)
        for bq in range(4):
            INDIC[bq, bq * 32 : (bq + 1) * 32, :] = 1.0
        _CACHE["tables"] = (W2INV17, WPOS2, np.ascontiguousarray(INDIC), W2INVP8)
    return _CACHE["tables"]


def _build(debug=False):
    key = ("nc", debug)
    if key in _CACHE:
        return _CACHE[key]

    from contextlib import ExitStack

    import concourse.bass as bass  # noqa: F401
    import concourse.tile as tile
    from concourse import bacc, mybir

    f32 = mybir.dt.float32
    AOT = mybir.AluOpType

    nc = bacc.Bacc(None, target_bir_lowering=False)
    x8 = nc.dram_tensor("x8", [BPC, ND, NT], f32, kind="ExternalInput")
    patts_d = nc.dram_tensor("patts_in", [NP, ND, NL], f32, kind="ExternalInput")
    w2inv_d = nc.dram_tensor("w2inv17", [K, NT], f32, kind="ExternalInput")
    wpos_d = nc.dram_tensor("wpos2", [128, 2, NT], f32, kind="ExternalInput")
    indic_d = nc.dram_tensor("indic", [4, 128, NL], f32, kind="ExternalInput")
    w2invp8_d = nc.dram_tensor("w2invp8", [8, NT], f32, kind="ExternalInput")
    out_d = nc.dram_tensor("out8", [BPC, NP, NT], f32, kind="ExternalOutput")
    if debug:
        dbg_lhsT = nc.dram_tensor("dbg_lhsT", [K, 128, NL], f32, kind="ExternalOutput")
        dbg_xw = nc.dram_tensor("dbg_xw", [2, K, NT], f32, kind="ExternalOutput")
        dbg_d = nc.dram_tensor("dbg_d", [2, 128, 2, NT], f32, kind="ExternalOutput")
        dbg_E = nc.dram_tensor("dbg_E", [4, 128, 2, NT + 1], f32, kind="ExternalOutput")

    with tile.TileContext(nc) as tc:
        with ExitStack() as ctx:
            persist = ctx.enter_context(tc.tile_pool(name="persist", bufs=1))
            dist_pool = ctx.enter_context(tc.tile_pool(name="dist", bufs=4))
            c_pool = ctx.enter_context(tc.tile_pool(name="cmin", bufs=4))
            psum_pool = ctx.enter_context(
                tc.tile_pool(name="psum", bufs=3, space="PSUM")
            )
            outp = ctx.enter_context(tc.tile_pool(name="outp", bufs=1))

            # lhsT free layout is (m, l): l contiguous so patts DMAs straight
            # from DRAM; the matmul reads the strided (K, m) slice at l=j
            lhsT = persist.tile([K, 128, NL], f32, name="lhsT")
            w2inv = persist.tile([K, NT], f32, name="w2inv")
            wpos = persist.tile([128, 2, NT], f32, name="wpos")
            inf2 = persist.tile([128, NT], f32, name="inf2")
            E0 = persist.tile([128, 2, NT + 1], f32, name="E0")
            E1 = persist.tile([128, 2, NT + 1], f32, name="E1")
            E = [E0, E1]

            xg0 = persist.tile([12, NT], f32, name="xg0")
            xg1 = persist.tile([12, NT], f32, name="xg1")
            xa8 = persist.tile([8, 3 * NT], f32, name="xa8")   # all 8 b, (d,t)
            x28 = persist.tile([8, NT], f32, name="x28")       # x2 for all 8 b
            xw0 = persist.tile([K, NT], f32, name="xw0")
            xw1 = persist.tile([K, NT], f32, name="xw1")
            w2invp8 = persist.tile([8, NT], f32, name="w2invp8")
            xg, xw = [xg0, xg1], [xw0, xw1]

            pp = persist.tile([NP, ND, NL], f32, name="pp")      # (p, d, l) natural
            ppsq = persist.tile([NP, ND, NL], f32, name="ppsq")
            p2e = persist.tile([NP, NL], f32, name="p2e")        # (p, l)

            # ---------------- input DMAs ----------------
            # startup latency matters: the xw (rhs) pipeline gates the first
            # matmul, so its inputs and compute are emitted first; the 1MB
            # wpos table is only needed by the output stage and loads later.
            actd = persist.tile([1, 1], f32, name="actd")
            nc.vector.memset(actd[:], 1.0)
            nc.scalar.sqrt(actd[:], actd[:])  # preload the Sqrt ACT table
            nc.scalar.dma_start(xa8[:], x8.rearrange("b d t -> b (d t)"))
            nc.sync.dma_start(w2invp8[:], w2invp8_d[:])
            nc.sync.dma_start(w2inv[:], w2inv_d[:])
            for h in range(2):
                bs = h * 4
                (nc.sync if h else nc.scalar).dma_start(
                    xg[h][:], x8[bs : bs + 4].rearrange("b d t -> (b d) t")
                )
            nc.scalar.dma_start(pp[:], patts_d[:])
            nc.sync.dma_start(lhsT[12:16, :, :], indic_d[:])

            # ---------------- rhs (xw) build ----------------
            # x2 for all 8 batches at partitions 0..7, then DMA into place
            nc.scalar.square(xa8[:], xa8[:])
            nc.vector.tensor_tensor(
                x28[:], xa8[:, 0:NT], xa8[:, NT : 2 * NT], op=AOT.add
            )
            nc.vector.tensor_tensor(
                x28[:], x28[:], xa8[:, 2 * NT : 3 * NT], op=AOT.add
            )
            nc.vector.tensor_tensor(x28[:], x28[:], w2invp8[:], op=AOT.mult)
            for h in range(2):
                nc.vector.tensor_tensor(
                    xw[h][0:12, :], xg[h][:], w2inv[0:12, :], op=AOT.mult
                )
                nc.scalar.dma_start(
                    xw[h][12:16, :], x28[h * 4 : h * 4 + 4, :]
                )
                nc.sync.dma_start(xw[h][16:17, :], w2inv_d[16:17, :])

            # ---------------- lhsT build ----------------
            # rows 12..15 (indic DMA) and 16 (p2e DMAs) are fully overwritten;
            # only the patts rows need zeroed off-diagonal blocks
            nc.gpsimd.memset(lhsT[0:12, :, :], 0.0)
            # p2 + eps row (row 16)
            nc.scalar.square(ppsq[:], pp[:])
            nc.vector.tensor_tensor(
                p2e[:], ppsq[:, 0, :], ppsq[:, 1, :], op=AOT.add
            )
            nc.vector.tensor_tensor(p2e[:], p2e[:], ppsq[:, 2, :], op=AOT.add)
            nc.vector.tensor_scalar_add(p2e[:], p2e[:], EPS)
            for bq in range(4):
                bs = bq * 32
                eng_a = nc.sync if bq % 2 == 0 else nc.scalar
                eng_b = nc.scalar if bq % 2 == 0 else nc.sync
                # patts block: (d, p, l) straight from DRAM, l contiguous
                eng_b.dma_start(
                    lhsT[bq * 3 : (bq + 1) * 3, bs : bs + 32, :],
                    patts_d.rearrange("p d l -> d p l"),
                )
                eng_a.dma_start(lhsT[16:17, bs : bs + 32, :], p2e[:])

            # ---------------- DP state init ----------------
            nc.vector.memset(inf2[:], INF)
            nc.vector.memset(E0[:, :, 0:1], INF)
            nc.vector.memset(E1[:, :, 0:1], INF)
            # wpos is first read ~180us in; load it behind the startup DMAs
            nc.scalar.dma_start(wpos[:], wpos_d[:])

            # ---------------- main loop over DP rows ----------------
            for j in range(NL):
                d3 = dist_pool.tile([128, 2, NT], f32, name="d3")
                for hh in range(2):
                    ps = psum_pool.tile([128, NT], f32, name="ps")
                    nc.tensor.matmul(
                        ps[:, 0:512],
                        lhsT[:, :, j],
                        xw[hh][:, 0:512],
                        start=True,
                        stop=True,
                    )
                    nc.tensor.matmul(
                        ps[:, 512:1024],
                        lhsT[:, :, j],
                        xw[hh][:, 512:1024],
                        start=True,
                        stop=True,
                    )
                    nc.scalar.sqrt(d3[:, hh, :], ps[:])
                if debug and j < 2:
                    nc.sync.dma_start(dbg_d[j], d3[:])

                Ecur, Eprev = E[j % 2], E[(j + 1) % 2]
                HS = NT // SEGS  # scan segment size
                segs = [(s * HS, (s + 1) * HS) for s in range(SEGS)]
                if j == 0:
                    for hh in range(2):
                        for s0, s1 in segs:
                            nc.vector.tensor_tensor_scan(
                                out=Ecur[:, hh, s0 + 1 : s1 + 1],
                                data0=inf2[:, s0:s1],
                                data1=d3[:, hh, s0:s1],
                                initial=0.0 if s0 == 0 else Ecur[:, hh, s0 : s0 + 1],
                                op0=AOT.min,
                                op1=AOT.add,
                            )
                        # row 0 is a cumsum (monotone in t), so row 1's
                        # window-min is just the shifted row; stash E0[0] in
                        # the edge slot so the shifted view is exact at t=0
                        nc.vector.tensor_copy(
                            out=Ecur[:, hh, 0:1], in_=Ecur[:, hh, 1:2]
                        )
                    if debug:
                        nc.sync.dma_start(dbg_E[0], Ecur[:])
                        nc.sync.dma_start(dbg_lhsT[:], lhsT[:])
                        nc.sync.dma_start(dbg_xw[0], xw[0][:])
                        nc.sync.dma_start(dbg_xw[1], xw[1][:])
                elif j == 1:
                    # min(E0[t], E0[t-1]) == E0[t-1] by monotonicity: use the
                    # shifted row directly, no window-min op
                    for hh in range(2):
                        nc.vector.tensor_tensor_scan(
                            out=Ecur[:, hh, 1 : NT + 1],
                            data0=Eprev[:, hh, 0:NT],
                            data1=d3[:, hh, :],
                            initial=INF,
                            op0=AOT.min,
                            op1=AOT.add,
                        )
                    # restore the INF edge for later rows reusing this buffer
                    nc.vector.memset(Eprev[:, :, 0:1], INF)
                else:
                    c3 = c_pool.tile([128, 2, NT], f32, name="c3")
                    for hh in range(2):
                        # window-min + scan both on DVE (the only engine that
                        # can run TensorTensor/scan); the two h-chains
                        # interleave to keep DVE busy
                        eng = nc.vector
                        for s0, s1 in segs:
                            eng.tensor_tensor(
                                c3[:, hh : hh + 1, s0:s1],
                                Eprev[:, hh : hh + 1, s0 + 1 : s1 + 1],
                                Eprev[:, hh : hh + 1, s0:s1],
                                op=AOT.min,
                            )
                            nc.vector.tensor_tensor_scan(
                                out=Ecur[:, hh, s0 + 1 : s1 + 1],
                                data0=c3[:, hh, s0:s1],
                                data1=d3[:, hh, s0:s1],
                                initial=INF if s0 == 0 else Ecur[:, hh, s0 : s0 + 1],
                                op0=AOT.min,
                                op1=AOT.add,
                            )
                    if debug and 1 <= j <= 3:
                        nc.sync.dma_start(dbg_E[j], Ecur[:])

            # ---------------- output ----------------
            # per-group rescale + store so h0's DMA overlaps h1's last scan;
            # each 512KB store is split across the two HWDGE queues
            Elast = E[(NL - 1) % 2]
            oth = outp.tile([128, 2, NT], f32, name="oth")
            of = out_d.rearrange("b p t -> (b p) t")
            for hh in range(2):
                nc.vector.tensor_tensor(
                    oth[:, hh, :],
                    Elast[:, hh, 1 : NT + 1],
                    wpos[:, hh, :],
                    op=AOT.mult,
                )
                rows = slice(hh * 128, (hh + 1) * 128)
                nc.sync.dma_start(of[rows, 0 : NT // 2], oth[:, hh, 0 : NT // 2])
                nc.scalar.dma_start(of[rows, NT // 2 : NT], oth[:, hh, NT // 2 : NT])

    nc.compile()
    _CACHE[key] = nc
    return nc


def _in_maps(x, patts):
    W2INV17, WPOS2, INDIC, W2INVP8 = _tables()
    x = np.ascontiguousarray(np.asarray(x, dtype=np.float32))
    patts = np.ascontiguousarray(np.asarray(patts, dtype=np.float32))
    maps = []
    for c in range(NCORES):
        maps.append(
            {
                "x8": np.ascontiguousarray(x[c * BPC : (c + 1) * BPC]),
                "patts_in": patts,
                "w2inv17": W2INV17,
                "wpos2": WPOS2,
                "indic": INDIC,
                "w2invp8": W2INVP8,
            }
        )
    return maps


def kernel(x, patts):
    nc = _build()
    from concourse.bass_utils import run_bass_kernel_spmd

    res = run_bass_kernel_spmd(
        nc, _in_maps(x, patts), core_ids=list(range(NCORES))
    )
    _CACHE["last_results"] = res
    out = np.concatenate([r["out8"] for r in res.results], axis=0)
    return out.astype(np.float32)



# revision 9
# speedup vs baseline: 1.2152x; 1.2152x over previous
"""Trainium2 Bass kernel for DTWFeatures.

Problem: x (64,3,1024), patts (32,3,32) -> out (64,32,1024)
  dist[b,p,l,t] = sqrt(max(|x[b,:,t]-patts[p,:,l]|^2, eps))
  DP:  D[l,t] = dist[l,t] + min(D[l-1,t], w*D[l,t-1], w*D[l-1,t-1])
  out[b,p,t] = D[L-1,t]

Strategy (8 cores, data-parallel over batch, 8 batches/core, 256 (b,p)
pairs/core = 2 groups of 128 partitions):
  * Rescale E[l,t] = D[l,t]*w^-(t-SHIFT): removes w from the recurrence.
  * Per DP row the recurrence is one DVE tensor_tensor_scan
    (op0=min, op1=add) with data0 = windowed min of the previous row.
  * Both 128-pair groups are processed by ONE 2049-wide scan per row: a
    boundary element with data1=1e30 blows the scan state up to ~1e30,
    which the next element's min() discards -- an exact chain reset.
  * E rows are stored bf16 (scan state stays fp32 internally), so the
    window-min TensorTensor runs in the DVE 2x_1p mode (2 elem/cycle).
    dist stays fp32.  Validated: L2 rel err ~3e-3 (tolerance 2e-2).
  * dist'^2 comes from a single K=33 fp32r matmul per 512-col chunk
    (fp32r = 1 cycle/row vs fp32's 4): rhs rows = x*-2w2inv (24),
    x2*w2inv (8), w2inv (1); lhsT = block-diag patts / batch indicators /
    p2+eps, one lhsT per group.  EPS=1.5e-2 floors d^2 against fp32r
    cancellation noise (HW sqrt(neg)=NaN).
  * ScalarE applies sqrt PSUM->SBUF; DVE is the bottleneck engine at
    ~105us busy (32 scans of 2049 + 30 bf16 window-mins).
"""

import os
import sys

if "/opt/trn_rl_repo" not in sys.path:
    sys.path.insert(0, "/opt/trn_rl_repo")
# the device path runs through jax's axon PJRT backend; make sure a
# harness-pinned JAX_PLATFORMS doesn't hide it (no-op if jax is already up)
if "jax" not in sys.modules and "axon" not in os.environ.get(
    "JAX_PLATFORMS", "axon"
):
    os.environ["JAX_PLATFORMS"] = "axon," + os.environ["JAX_PLATFORMS"]

import numpy as np

NB, ND, NP, NL, NT = 64, 3, 32, 32, 1024   # batch, xdim, n_patts, l_patts, T
NCORES = 8
BPC = NB // NCORES                     # 8 batches per core
RHO = 0.1
W = RHO ** (1.0 / NL)
SHIFT = 512.0
EPS = 1.5e-2                           # floors d^2 against fp32r noise
INF = 1.0e30
LARGE = 1.0e30                         # scan chain-reset boundary value
K = 33                                 # matmul contraction rows
NT2 = 2 * NT + 1                       # merged scan length (1024+1+1024)

_CACHE = {}


def _tables():
    if "tables" not in _CACHE:
        import ml_dtypes

        t = np.arange(NT, dtype=np.float64)
        w2inv = (W ** (-2.0 * (t - SHIFT))).astype(np.float32)
        wpos = (W ** (t - SHIFT)).astype(np.float32)
        W2INV24 = np.ascontiguousarray(np.tile(w2inv[None, :], (24, 1)) * -2.0)
        W2INVP8 = np.ascontiguousarray(np.tile(w2inv[None, :], (8, 1)))
        W2INV1 = np.ascontiguousarray(w2inv[None, :])
        WPOS2 = np.ascontiguousarray(np.tile(wpos[None, None, :], (128, 2, 1)))
        # E init row: INF at the two edge slots, per-group layout
        # [edge0 | 1024 h0 outs | edge1 | 1024 h1 outs]
        EINIT = np.zeros((128, NT2 + 1), np.float32)
        EINIT[:, 0] = INF
        EINIT[:, NT + 1] = INF
        EINIT = EINIT.astype(ml_dtypes.bfloat16)
        EINF1 = np.full((128, 1), INF, np.float32).astype(ml_dtypes.bfloat16)
        # scan data0 for row 0: INF (cumsum) with a 0 at the h1 restart slot
        INF2 = np.full((128, NT2), INF, np.float32)
        INF2[:, NT + 1] = 0.0
        _CACHE["tables"] = (W2INV24, W2INVP8, W2INV1, WPOS2, EINIT, EINF1,
                            np.ascontiguousarray(INF2))
    return _CACHE["tables"]


def _lhbase(patts):
    """Block-diagonal lhsT rows 0:32 per group: raw patts blocks (rows
    0:24, the -2 scale lives in the rhs) + batch indicators (rows 24:32)."""
    lhb = np.zeros((2, 32, 128, NL), np.float32)
    pdl = np.transpose(patts, (1, 0, 2))  # (d, p, l)
    for h in range(2):
        for bl in range(4):
            r = 12 * h + 3 * bl
            lhb[h, r : r + 3, 32 * bl : 32 * (bl + 1), :] = pdl
            lhb[h, 24 + 4 * h + bl, 32 * bl : 32 * (bl + 1), :] = 1.0
    return lhb


def _build():
    if "nc" in _CACHE:
        return _CACHE["nc"]

    from contextlib import ExitStack

    import concourse.bass as bass  # noqa: F401
    import concourse.tile as tile
    from concourse import bacc, mybir

    f32 = mybir.dt.float32
    f32r = mybir.dt.float32r
    bf16 = mybir.dt.bfloat16
    AOT = mybir.AluOpType

    nc = bacc.Bacc(None, target_bir_lowering=False)
    x8 = nc.dram_tensor("x8", [BPC, ND, NT], f32, kind="ExternalInput")
    patts_f = nc.dram_tensor("patts_f", [NP, ND, NL], f32, kind="ExternalInput")
    lhbase_d = nc.dram_tensor("lhbase", [2, 32, 128, NL], f32r, kind="ExternalInput")
    w2inv24_d = nc.dram_tensor("w2inv24", [24, NT], f32, kind="ExternalInput")
    w2invp8_d = nc.dram_tensor("w2invp8", [8, NT], f32, kind="ExternalInput")
    w2inv1_d = nc.dram_tensor("w2inv1", [1, NT], f32r, kind="ExternalInput")
    einit_d = nc.dram_tensor("einit", [128, NT2 + 1], bf16, kind="ExternalInput")
    einf1_d = nc.dram_tensor("einf1", [128, 1], bf16, kind="ExternalInput")
    inf2_d = nc.dram_tensor("inf2d", [128, NT2], f32, kind="ExternalInput")
    wpos_d = nc.dram_tensor("wpos2", [128, 2, NT], f32, kind="ExternalInput")
    out_d = nc.dram_tensor("out8", [BPC, NP, NT], f32, kind="ExternalOutput")

    with tile.TileContext(nc) as tc:
        with ExitStack() as ctx:
            persist = ctx.enter_context(tc.tile_pool(name="persist", bufs=1))
            c_pool = ctx.enter_context(tc.tile_pool(name="cmin", bufs=2))
            psum_pool = ctx.enter_context(
                tc.tile_pool(name="psum", bufs=2, space="PSUM")
            )

            # lhsT per group: free layout (m, l), l contiguous for patts DMA
            lh = [persist.tile([K, 128, NL], f32r, name=f"lh{h}") for h in range(2)]
            xw = persist.tile([K, NT], f32r, name="xw")
            w2inv24 = persist.tile([24, NT], f32, name="w2inv24")
            w2invp8 = persist.tile([8, NT], f32, name="w2invp8")
            xg = persist.tile([24, NT], f32, name="xg")
            xa8 = persist.tile([8, 3 * NT], f32, name="xa8")
            x2sum = persist.tile([8, NT], f32, name="x2sum")
            x2w = persist.tile([8, NT], f32r, name="x2w")
            pp = persist.tile([NP, ND, NL], f32, name="pp")
            ppsq = persist.tile([NP, ND, NL], f32, name="ppsq")
            p2t = persist.tile([NP, NL], f32, name="p2t")
            p2e = persist.tile([NP, NL], f32r, name="p2e")
            wpos = persist.tile([128, 2, NT], f32, name="wpos")
            inf2 = persist.tile([128, NT2], f32, name="inf2")
            einf = persist.tile([128, 1], bf16, name="einf")
            E0 = persist.tile([128, NT2 + 1], bf16, name="E0")
            E1 = persist.tile([128, NT2 + 1], bf16, name="E1")
            E = [E0, E1]
            d3r = [persist.tile([128, NT2], f32, name=f"d3_{i}") for i in range(3)]
            oth = persist.tile([128, 2, NT], f32, name="oth")

            # ---------------- startup ----------------
            actd = persist.tile([1, 1], f32, name="actd")
            nc.vector.memset(actd[:], 1.0)
            nc.scalar.sqrt(actd[:], actd[:])  # preload the Sqrt ACT table

            nc.scalar.dma_start(xa8[:], x8.rearrange("b d t -> b (d t)"))
            nc.sync.dma_start(xg[:], x8.rearrange("b d t -> (b d) t"))
            nc.sync.dma_start(w2inv24[:], w2inv24_d[:])
            nc.sync.dma_start(w2invp8[:], w2invp8_d[:])
            nc.scalar.dma_start(pp[:], patts_f[:])
            nc.sync.dma_start(xw[32:33, :], w2inv1_d[:])

            nc.sync.dma_start(lh[0][0:32, :, :], lhbase_d[0])
            nc.scalar.dma_start(lh[1][0:32, :, :], lhbase_d[1])
            nc.gpsimd.dma_start(einf[:], einf1_d[:])
            nc.gpsimd.dma_start(E0[:], einit_d[:])
            nc.gpsimd.dma_start(E1[:], einit_d[:])
            nc.gpsimd.dma_start(inf2[:], inf2_d[:])

            # rhs (xw) build: squares split 3x to pipeline ACT with DVE adds
            for s in range(3):
                nc.scalar.square(
                    xa8[:, s * NT : (s + 1) * NT], xa8[:, s * NT : (s + 1) * NT]
                )
            nc.vector.tensor_tensor(xw[0:24, :], xg[:], w2inv24[:], op=AOT.mult)
            nc.vector.tensor_tensor(
                x2sum[:], xa8[:, 0:NT], xa8[:, NT : 2 * NT], op=AOT.add
            )
            nc.vector.tensor_tensor(
                x2sum[:], x2sum[:], xa8[:, 2 * NT : 3 * NT], op=AOT.add
            )
            nc.vector.tensor_tensor(x2w[:], x2sum[:], w2invp8[:], op=AOT.mult)
            nc.scalar.dma_start(xw[24:32, :], x2w[:])

            # p2 + eps row (row 32)
            nc.scalar.square(ppsq[:], pp[:])
            nc.vector.tensor_tensor(
                p2t[:], ppsq[:, 0, :], ppsq[:, 1, :], op=AOT.add
            )
            nc.vector.tensor_tensor(p2t[:], p2t[:], ppsq[:, 2, :], op=AOT.add)
            nc.vector.tensor_scalar_add(p2e[:], p2t[:], EPS)
            for h in range(2):
                for bl in range(4):
                    eng = nc.sync if (h + bl) % 2 == 0 else nc.scalar
                    eng.dma_start(
                        lh[h][32:33, 32 * bl : 32 * (bl + 1), :], p2e[:]
                    )

            # ---------------- DP state init ----------------
            for i in range(3):
                nc.vector.memset(d3r[i][:, NT : NT + 1], LARGE)  # chain reset

            # ---------------- main loop over DP rows ----------------
            for j in range(NL):
                ps = psum_pool.tile([128, 2, NT], f32, name="ps")
                d3 = d3r[j % 3]
                for h in range(2):
                    nc.tensor.matmul(
                        ps[:, h, 0:512], lh[h][:, :, j], xw[:, 0:512],
                        start=True, stop=True,
                    )
                    nc.tensor.matmul(
                        ps[:, h, 512:1024], lh[h][:, :, j], xw[:, 512:1024],
                        start=True, stop=True,
                    )
                nc.scalar.sqrt(d3[:, 0:NT], ps[:, 0, :])
                nc.scalar.sqrt(d3[:, NT + 1 : NT2], ps[:, 1, :])

                Ecur, Eprev = E[j % 2], E[(j + 1) % 2]
                if j == 0:
                    nc.vector.tensor_tensor_scan(
                        out=Ecur[:, 1 : NT2 + 1],
                        data0=inf2[:, 0:NT2],
                        data1=d3[:, 0:NT2],
                        initial=0.0,
                        op0=AOT.min,
                        op1=AOT.add,
                    )
                    # stash E0[t=0] per group into the edge slots so row 1's
                    # shifted-data0 view (cumsum monotonicity shortcut) is
                    # exact at t=0
                    nc.vector.tensor_copy(out=Ecur[:, 0:1], in_=Ecur[:, 1:2])
                    nc.vector.tensor_copy(
                        out=Ecur[:, NT + 1 : NT + 2], in_=Ecur[:, NT + 2 : NT + 3]
                    )
                elif j == 1:
                    # min(E0[t], E0[t-1]) == E0[t-1] by monotonicity
                    nc.vector.tensor_tensor_scan(
                        out=Ecur[:, 1 : NT2 + 1],
                        data0=Eprev[:, 0:NT2],
                        data1=d3[:, 0:NT2],
                        initial=INF,
                        op0=AOT.min,
                        op1=AOT.add,
                    )
                    # restore the INF edges for row 2's buffer reuse
                    nc.vector.tensor_copy(out=Eprev[:, 0:1], in_=einf[:])
                    nc.vector.tensor_copy(
                        out=Eprev[:, NT + 1 : NT + 2], in_=einf[:]
                    )
                else:
                    c3 = c_pool.tile([128, NT2], bf16, name="c3")
                    nc.vector.tensor_tensor(
                        c3[:], Eprev[:, 1 : NT2 + 1], Eprev[:, 0:NT2], op=AOT.min
                    )
                    nc.vector.tensor_tensor_scan(
                        out=Ecur[:, 1 : NT2 + 1],
                        data0=c3[:],
                        data1=d3[:, 0:NT2],
                        initial=INF,
                        op0=AOT.min,
                        op1=AOT.add,
                    )
                if j == 0:
                    # 1MB table only needed by the output stage; load it
                    # behind the startup DMAs
                    nc.gpsimd.dma_start(wpos[:], wpos_d[:])

            # ---------------- output ----------------
            Elast = E[(NL - 1) % 2]
            of = out_d.rearrange("b p t -> (b p) t")
            for h in range(2):
                lo = 1 if h == 0 else NT + 2
                nc.vector.tensor_tensor(
                    oth[:, h, :],
                    Elast[:, lo : lo + NT],
                    wpos[:, h, :],
                    op=AOT.mult,
                )
                rows = slice(h * 128, (h + 1) * 128)
                nc.sync.dma_start(of[rows, 0 : NT // 2], oth[:, h, 0 : NT // 2])
                nc.scalar.dma_start(of[rows, NT // 2 : NT], oth[:, h, NT // 2 : NT])

    nc.compile()
    _CACHE["nc"] = nc
    return nc


def _in_maps(x, patts):
    W2INV24, W2INVP8, W2INV1, WPOS2, EINIT, EINF1, INF2 = _tables()
    x = np.ascontiguousarray(np.asarray(x, dtype=np.float32))
    patts = np.ascontiguousarray(np.asarray(patts, dtype=np.float32))
    lhb = _lhbase(patts)
    maps = []
    for c in range(NCORES):
        maps.append(
            {
                "x8": np.ascontiguousarray(x[c * BPC : (c + 1) * BPC]),
                "patts_f": patts,
                "lhbase": lhb,
                "w2inv24": W2INV24,
                "w2invp8": W2INVP8,
                "w2inv1": W2INV1,
                "einit": EINIT,
                "einf1": EINF1,
                "inf2d": INF2,
                "wpos2": WPOS2,
            }
        )
    return maps


def kernel(x, patts):
    nc = _build()
    from concourse.bass_utils import run_bass_kernel_spmd

    res = run_bass_kernel_spmd(
        nc, _in_maps(x, patts), core_ids=list(range(NCORES))
    )
    _CACHE["last_results"] = res
    out = np.concatenate([r["out8"] for r in res.results], axis=0)
    return out.astype(np.float32)


# revision 13
# speedup vs baseline: 1.2259x; 1.0088x over previous
"""Trainium2 Bass kernel for DTWFeatures.

Problem: x (64,3,1024), patts (32,3,32) -> out (64,32,1024)
  dist[b,p,l,t] = sqrt(max(|x[b,:,t]-patts[p,:,l]|^2, eps))
  DP:  D[l,t] = dist[l,t] + min(D[l-1,t], w*D[l,t-1], w*D[l-1,t-1])
  out[b,p,t] = D[L-1,t]

Strategy (8 cores, data-parallel over batch, 8 batches/core, 256 (b,p)
pairs/core = 2 groups of 128 partitions):
  * Rescale E[l,t] = D[l,t]*w^-(t-SHIFT): removes w from the recurrence.
  * Per DP row the recurrence is one DVE tensor_tensor_scan
    (op0=min, op1=add) with data0 = windowed min of the previous row.
  * Both 128-pair groups are processed by ONE 2049-wide scan per row: a
    boundary element with data1=1e30 blows the scan state up to ~1e30,
    which the next element's min() discards -- an exact chain reset.
  * E rows are stored bf16 (scan state stays fp32 internally), so the
    window-min TensorTensor runs in the DVE 2x_1p mode (2 elem/cycle).
    dist stays fp32.  Validated: L2 rel err ~3e-3 (tolerance 2e-2).
  * dist'^2 comes from a single K=33 fp32r matmul per 512-col chunk
    (fp32r = 1 cycle/row vs fp32's 4): rhs rows = x*-2w2inv (24),
    x2*w2inv (8), w2inv (1); lhsT = block-diag patts / batch indicators /
    p2+eps, one lhsT per group.  EPS=1.5e-2 floors d^2 against fp32r
    cancellation noise (HW sqrt(neg)=NaN).
  * ScalarE applies sqrt PSUM->SBUF; DVE is the bottleneck engine at
    ~105us busy (32 scans of 2049 + 30 bf16 window-mins).
"""

import os
import sys

if "/opt/trn_rl_repo" not in sys.path:
    sys.path.insert(0, "/opt/trn_rl_repo")
# the device path runs through jax's axon PJRT backend; make sure a
# harness-pinned JAX_PLATFORMS doesn't hide it (no-op if jax is already up)
if "jax" not in sys.modules and "axon" not in os.environ.get(
    "JAX_PLATFORMS", "axon"
):
    os.environ["JAX_PLATFORMS"] = "axon," + os.environ["JAX_PLATFORMS"]

import numpy as np

NB, ND, NP, NL, NT = 64, 3, 32, 32, 1024   # batch, xdim, n_patts, l_patts, T
NCORES = 8
BPC = NB // NCORES                     # 8 batches per core
RHO = 0.1
W = RHO ** (1.0 / NL)
SHIFT = 512.0
EPS = 1.5e-2                           # floors d^2 against fp32r noise
INF = 1.0e30
LARGE = 1.0e30                         # scan chain-reset boundary value
K = 33                                 # matmul contraction rows
NT2 = 2 * NT + 1                       # merged scan length (1024+1+1024)

_CACHE = {}


def _tables():
    if "tables" not in _CACHE:
        import ml_dtypes

        t = np.arange(NT, dtype=np.float64)
        w2inv = (W ** (-2.0 * (t - SHIFT))).astype(np.float32)
        wpos = (W ** (t - SHIFT)).astype(np.float32)
        W2INV24 = np.ascontiguousarray(np.tile(w2inv[None, :], (24, 1)) * -2.0)
        W2INVP8 = np.ascontiguousarray(np.tile(w2inv[None, :], (8, 1)))
        W2INV1 = np.ascontiguousarray(w2inv[None, :])
        WPOS2 = np.ascontiguousarray(np.tile(wpos[None, None, :], (128, 2, 1)))
        # E init row: INF at the two edge slots, per-group layout
        # [edge0 | 1024 h0 outs | edge1 | 1024 h1 outs]
        EINIT = np.zeros((128, NT2 + 1), np.float32)
        EINIT[:, 0] = INF
        EINIT[:, NT + 1] = INF
        EINIT = EINIT.astype(ml_dtypes.bfloat16)
        EINF1 = np.full((128, 1), INF, np.float32).astype(ml_dtypes.bfloat16)
        # scan data0 for row 0: INF (cumsum) with a 0 at the h1 restart slot
        INF2 = np.full((128, NT2), INF, np.float32)
        INF2[:, NT + 1] = 0.0
        _CACHE["tables"] = (W2INV24, W2INVP8, W2INV1, WPOS2, EINIT, EINF1,
                            np.ascontiguousarray(INF2))
    return _CACHE["tables"]


def _lhbase(patts):
    """Block-diagonal lhsT rows 0:32 per group: raw patts blocks (rows
    0:24, the -2 scale lives in the rhs) + batch indicators (rows 24:32)."""
    lhb = np.zeros((2, 32, 128, NL), np.float32)
    pdl = np.transpose(patts, (1, 0, 2))  # (d, p, l)
    for h in range(2):
        for bl in range(4):
            r = 12 * h + 3 * bl
            lhb[h, r : r + 3, 32 * bl : 32 * (bl + 1), :] = pdl
            lhb[h, 24 + 4 * h + bl, 32 * bl : 32 * (bl + 1), :] = 1.0
    return lhb


def _build():
    if "nc" in _CACHE:
        return _CACHE["nc"]

    from contextlib import ExitStack

    import concourse.bass as bass  # noqa: F401
    import concourse.tile as tile
    from concourse import bacc, mybir

    f32 = mybir.dt.float32
    f32r = mybir.dt.float32r
    bf16 = mybir.dt.bfloat16
    AOT = mybir.AluOpType

    nc = bacc.Bacc(None, target_bir_lowering=False)
    x8 = nc.dram_tensor("x8", [BPC, ND, NT], f32, kind="ExternalInput")
    patts_f = nc.dram_tensor("patts_f", [NP, ND, NL], f32, kind="ExternalInput")
    lhbase_d = nc.dram_tensor("lhbase", [2, 32, 128, NL], f32r, kind="ExternalInput")
    w2inv24_d = nc.dram_tensor("w2inv24", [24, NT], f32, kind="ExternalInput")
    w2invp8_d = nc.dram_tensor("w2invp8", [8, NT], f32, kind="ExternalInput")
    w2inv1_d = nc.dram_tensor("w2inv1", [1, NT], f32r, kind="ExternalInput")
    einf1_d = nc.dram_tensor("einf1", [128, 1], bf16, kind="ExternalInput")
    wpos_d = nc.dram_tensor("wpos2", [128, 2, NT], f32, kind="ExternalInput")
    out_d = nc.dram_tensor("out8", [BPC, NP, NT], f32, kind="ExternalOutput")

    with tile.TileContext(nc) as tc:
        with ExitStack() as ctx:
            persist = ctx.enter_context(tc.tile_pool(name="persist", bufs=1))
            c_pool = ctx.enter_context(tc.tile_pool(name="cmin", bufs=2))
            psum_pool = ctx.enter_context(
                tc.tile_pool(name="psum", bufs=2, space="PSUM")
            )

            # lhsT per group: free layout (m, l), l contiguous for patts DMA
            lh = [persist.tile([K, 128, NL], f32r, name=f"lh{h}") for h in range(2)]
            xw = persist.tile([K, NT], f32r, name="xw")
            w2inv24 = persist.tile([24, NT], f32, name="w2inv24")
            w2invp8 = persist.tile([8, NT], f32, name="w2invp8")
            xg = persist.tile([24, NT], f32, name="xg")
            xa8 = persist.tile([8, 3 * NT], f32, name="xa8")
            x2sum = persist.tile([8, NT], f32, name="x2sum")
            x2w = persist.tile([8, NT], f32r, name="x2w")
            pp = persist.tile([NP, ND, NL], f32, name="pp")
            ppsq = persist.tile([NP, ND, NL], f32, name="ppsq")
            p2t = persist.tile([NP, NL], f32, name="p2t")
            p2e = persist.tile([NP, NL], f32r, name="p2e")
            wpos = persist.tile([128, 2, NT], f32, name="wpos")
            inf2 = persist.tile([128, NT2], f32, name="inf2")
            einf = persist.tile([128, 1], bf16, name="einf")
            E0 = persist.tile([128, NT2 + 1], bf16, name="E0")
            E1 = persist.tile([128, NT2 + 1], bf16, name="E1")
            E = [E0, E1]
            d3r = [persist.tile([128, NT2], f32, name=f"d3_{i}") for i in range(3)]
            oth = persist.tile([128, 2, NT], f32, name="oth")

            # ---------------- startup ----------------
            # critical path to the first scan: xa8 DMA -> squares -> x2sum
            # adds -> x2w -> xw[24:32] DMA -> matmuls -> sqrts.  Everything
            # else (p2e chain, lhbase, tables) is ordered to stay off it.
            actd = persist.tile([1, 1], f32, name="actd")
            nc.vector.memset(actd[:], 1.0)
            nc.scalar.sqrt(actd[:], actd[:])  # preload the Sqrt ACT table

            nc.scalar.dma_start(xa8[:], x8.rearrange("b d t -> b (d t)"))
            nc.sync.dma_start(pp[:], patts_f[:])
            nc.sync.dma_start(xg[:], x8.rearrange("b d t -> (b d) t"))
            nc.sync.dma_start(w2inv24[:], w2inv24_d[:])
            nc.sync.dma_start(w2invp8[:], w2invp8_d[:])
            nc.sync.dma_start(xw[32:33, :], w2inv1_d[:])

            nc.gpsimd.dma_start(einf[:], einf1_d[:])
            nc.gpsimd.dma_start(lh[0][0:32, :, :], lhbase_d[0])
            nc.gpsimd.dma_start(lh[1][0:32, :, :], lhbase_d[1])

            # p2 + eps row (row 32) -- small, goes first on ACT/DVE
            nc.scalar.square(ppsq[:], pp[:])
            # rhs (xw) build: squares split 3x to pipeline ACT with DVE adds
            for s in range(3):
                nc.scalar.square(
                    xa8[:, s * NT : (s + 1) * NT], xa8[:, s * NT : (s + 1) * NT]
                )
            nc.vector.tensor_tensor(
                p2t[:], ppsq[:, 0, :], ppsq[:, 1, :], op=AOT.add
            )
            nc.vector.tensor_tensor(p2t[:], p2t[:], ppsq[:, 2, :], op=AOT.add)
            nc.vector.tensor_scalar_add(p2e[:], p2t[:], EPS)
            for h in range(2):
                for bl in range(4):
                    eng = nc.sync if (h + bl) % 2 == 0 else nc.scalar
                    eng.dma_start(
                        lh[h][32:33, 32 * bl : 32 * (bl + 1), :], p2e[:]
                    )

            nc.vector.tensor_tensor(xw[0:24, :], xg[:], w2inv24[:], op=AOT.mult)
            nc.vector.tensor_tensor(
                x2sum[:], xa8[:, 0:NT], xa8[:, NT : 2 * NT], op=AOT.add
            )
            nc.vector.tensor_tensor(
                x2sum[:], x2sum[:], xa8[:, 2 * NT : 3 * NT], op=AOT.add
            )
            nc.vector.tensor_tensor(x2w[:], x2sum[:], w2invp8[:], op=AOT.mult)
            nc.scalar.dma_start(xw[24:32, :], x2w[:])

            # ---------------- DP state init (DVE fills its idle window) ----
            nc.vector.memset(inf2[:], INF)
            nc.vector.memset(inf2[:, NT + 1 : NT + 2], 0.0)  # h1 cumsum restart
            # E1 edge slots; E0's edges are only read via the j==0 stash
            nc.vector.tensor_copy(out=E1[:, 0:1], in_=einf[:])
            nc.vector.tensor_copy(out=E1[:, NT + 1 : NT + 2], in_=einf[:])
            for i in range(3):
                nc.vector.memset(d3r[i][:, NT : NT + 1], LARGE)  # chain reset

            # ---------------- main loop over DP rows ----------------
            for j in range(NL):
                ps = psum_pool.tile([128, 2, NT], f32, name="ps")
                d3 = d3r[j % 3]
                for h in range(2):
                    nc.tensor.matmul(
                        ps[:, h, 0:512], lh[h][:, :, j], xw[:, 0:512],
                        start=True, stop=True,
                    )
                    nc.tensor.matmul(
                        ps[:, h, 512:1024], lh[h][:, :, j], xw[:, 512:1024],
                        start=True, stop=True,
                    )
                nc.scalar.sqrt(d3[:, 0:NT], ps[:, 0, :])
                nc.scalar.sqrt(d3[:, NT + 1 : NT2], ps[:, 1, :])

                Ecur, Eprev = E[j % 2], E[(j + 1) % 2]
                if j == 0:
                    # split per group: the h0 cumsum starts as soon as h0's
                    # sqrt lands instead of waiting for both halves
                    nc.vector.tensor_tensor_scan(
                        out=Ecur[:, 1 : NT + 1],
                        data0=inf2[:, 0:NT],
                        data1=d3[:, 0:NT],
                        initial=0.0,
                        op0=AOT.min,
                        op1=AOT.add,
                    )
                    nc.vector.tensor_tensor_scan(
                        out=Ecur[:, NT + 2 : NT2 + 1],
                        data0=inf2[:, 0:NT],
                        data1=d3[:, NT + 1 : NT2],
                        initial=0.0,
                        op0=AOT.min,
                        op1=AOT.add,
                    )
                    # stash E0[t=0] per group into the edge slots so row 1's
                    # shifted-data0 view (cumsum monotonicity shortcut) is
                    # exact at t=0
                    nc.vector.tensor_copy(out=Ecur[:, 0:1], in_=Ecur[:, 1:2])
                    nc.vector.tensor_copy(
                        out=Ecur[:, NT + 1 : NT + 2], in_=Ecur[:, NT + 2 : NT + 3]
                    )
                elif j == 1:
                    # min(E0[t], E0[t-1]) == E0[t-1] by monotonicity
                    nc.vector.tensor_tensor_scan(
                        out=Ecur[:, 1 : NT2 + 1],
                        data0=Eprev[:, 0:NT2],
                        data1=d3[:, 0:NT2],
                        initial=INF,
                        op0=AOT.min,
                        op1=AOT.add,
                    )
                    # restore the INF edges for row 2's buffer reuse
                    nc.vector.tensor_copy(out=Eprev[:, 0:1], in_=einf[:])
                    nc.vector.tensor_copy(
                        out=Eprev[:, NT + 1 : NT + 2], in_=einf[:]
                    )
                elif j < NL - 1:
                    c3 = c_pool.tile([128, NT2], bf16, name="c3")
                    nc.vector.tensor_tensor(
                        c3[:], Eprev[:, 1 : NT2 + 1], Eprev[:, 0:NT2], op=AOT.min
                    )
                    nc.vector.tensor_tensor_scan(
                        out=Ecur[:, 1 : NT2 + 1],
                        data0=c3[:],
                        data1=d3[:, 0:NT2],
                        initial=INF,
                        op0=AOT.min,
                        op1=AOT.add,
                    )
                else:
                    # last row split per group: h0's output rescale + store
                    # overlap h1's scan
                    of = out_d.rearrange("b p t -> (b p) t")
                    c3 = c_pool.tile([128, NT2], bf16, name="c3")
                    nc.vector.tensor_tensor(
                        c3[:], Eprev[:, 1 : NT2 + 1], Eprev[:, 0:NT2], op=AOT.min
                    )
                    for h in range(2):
                        lo = 1 if h == 0 else NT + 2
                        nc.vector.tensor_tensor_scan(
                            out=Ecur[:, lo : lo + NT],
                            data0=c3[:, lo - 1 : lo - 1 + NT],
                            data1=d3[:, lo - 1 : lo - 1 + NT],
                            initial=INF,
                            op0=AOT.min,
                            op1=AOT.add,
                        )
                        nc.vector.tensor_tensor(
                            oth[:, h, :],
                            Ecur[:, lo : lo + NT],
                            wpos[:, h, :],
                            op=AOT.mult,
                        )
                        rows = slice(h * 128, (h + 1) * 128)
                        nc.sync.dma_start(
                            of[rows, 0 : NT // 2], oth[:, h, 0 : NT // 2]
                        )
                        nc.scalar.dma_start(
                            of[rows, NT // 2 : NT], oth[:, h, NT // 2 : NT]
                        )
                if j == 0:
                    # 1MB table only needed by the output stage; load it
                    # behind the startup DMAs
                    nc.gpsimd.dma_start(wpos[:], wpos_d[:])

    nc.compile()
    _CACHE["nc"] = nc
    return nc


def _in_maps(x, patts):
    W2INV24, W2INVP8, W2INV1, WPOS2, EINIT, EINF1, INF2 = _tables()
    x = np.ascontiguousarray(np.asarray(x, dtype=np.float32))
    patts = np.ascontiguousarray(np.asarray(patts, dtype=np.float32))
    lhb = _lhbase(patts)
    maps = []
    for c in range(NCORES):
        maps.append(
            {
                "x8": np.ascontiguousarray(x[c * BPC : (c + 1) * BPC]),
                "patts_f": patts,
                "lhbase": lhb,
                "w2inv24": W2INV24,
                "w2invp8": W2INVP8,
                "w2inv1": W2INV1,
                "einit": EINIT,
                "einf1": EINF1,
                "inf2d": INF2,
                "wpos2": WPOS2,
            }
        )
    return maps


def kernel(x, patts):
    nc = _build()
    from concourse.bass_utils import run_bass_kernel_spmd

    res = run_bass_kernel_spmd(
        nc, _in_maps(x, patts), core_ids=list(range(NCORES))
    )
    _CACHE["last_results"] = res
    out = np.concatenate([r["out8"] for r in res.results], axis=0)
    return out.astype(np.float32)


# revision 25
# speedup vs baseline: 1.2564x; 1.0249x over previous
"""Trainium2 Bass kernel for DTWFeatures.

Problem: x (64,3,1024), patts (32,3,32) -> out (64,32,1024)
  dist[b,p,l,t] = sqrt(max(|x[b,:,t]-patts[p,:,l]|^2, eps))
  DP:  D[l,t] = dist[l,t] + min(D[l-1,t], w*D[l,t-1], w*D[l-1,t-1])
  out[b,p,t] = D[L-1,t]

Strategy (8 cores, data-parallel over batch, 8 batches/core, 256 (b,p)
pairs/core = 2 groups of 128 partitions):
  * Rescale E[l,t] = D[l,t]*w^-(t-SHIFT): removes w from the recurrence.
  * Per DP row the recurrence is one DVE tensor_tensor_scan
    (op0=min, op1=add) with data0 = windowed min of the previous row.
  * Both 128-pair groups are processed by ONE 2049-wide scan per row: a
    boundary element with data1=1e30 blows the scan state up to ~1e30,
    which the next element's min() discards -- an exact chain reset.
  * E rows are stored bf16 (scan state stays fp32 internally), so the
    window-min TensorTensor runs in the DVE 2x_1p mode (2 elem/cycle).
    dist stays fp32.  Validated: L2 rel err ~3e-3 (tolerance 2e-2).
  * dist'^2 comes from a single K=33 fp32r matmul per 512-col chunk
    (fp32r = 1 cycle/row vs fp32's 4): rhs rows = x*-2w2inv (24),
    x2*w2inv (8), w2inv (1); lhsT = block-diag patts / batch indicators /
    p2+eps, one lhsT per group.  EPS=1.5e-2 floors d^2 against fp32r
    cancellation noise (HW sqrt(neg)=NaN).
  * ScalarE applies sqrt PSUM->SBUF; DVE is the bottleneck engine at
    ~105us busy (32 scans of 2049 + 30 bf16 window-mins).
"""

import os
import sys

if "/opt/trn_rl_repo" not in sys.path:
    sys.path.insert(0, "/opt/trn_rl_repo")
# the device path runs through jax's axon PJRT backend; make sure a
# harness-pinned JAX_PLATFORMS doesn't hide it (no-op if jax is already up)
if "jax" not in sys.modules and "axon" not in os.environ.get(
    "JAX_PLATFORMS", "axon"
):
    os.environ["JAX_PLATFORMS"] = "axon," + os.environ["JAX_PLATFORMS"]

import numpy as np

NB, ND, NP, NL, NT = 64, 3, 32, 32, 1024   # batch, xdim, n_patts, l_patts, T
NCORES = 8
BPC = NB // NCORES                     # 8 batches per core
RHO = 0.1
W = RHO ** (1.0 / NL)
SHIFT = 512.0
EPS = 1.5e-2                           # floors d^2 against fp32r noise
INF = 1.0e30
LARGE = 1.0e30                         # scan chain-reset boundary value
K = 33                                 # matmul contraction rows
NT2 = 2 * NT + 1                       # merged scan length (1024+1+1024)

_CACHE = {}


def _tables():
    if "tables" not in _CACHE:
        import ml_dtypes

        t = np.arange(NT, dtype=np.float64)
        w2inv = (W ** (-2.0 * (t - SHIFT))).astype(np.float32)
        wpos = (W ** (t - SHIFT)).astype(np.float32)
        W2INV24 = np.ascontiguousarray(np.tile(w2inv[None, :], (24, 1)) * -2.0)
        W2INVP8 = np.ascontiguousarray(np.tile(w2inv[None, :], (8, 1)))
        W2INV1 = np.ascontiguousarray(w2inv[None, :])
        WPOS2 = np.ascontiguousarray(np.tile(wpos[None, None, :], (128, 2, 1)))
        EINF1 = np.full((128, 1), INF, np.float32).astype(ml_dtypes.bfloat16)
        _CACHE["tables"] = (W2INV24, W2INVP8, W2INV1, WPOS2, EINF1)
    return _CACHE["tables"]


def _lhbase(patts):
    """Full stationary lhsT per group: block-diagonal raw patts (rows 0:24,
    the -2 scale lives in the rhs), batch indicators (rows 24:32), and the
    p^2+eps row (row 32, paired with the rhs w2inv row)."""
    lhb = np.zeros((2, K, 128, NL), np.float32)
    pdl = np.transpose(patts, (1, 0, 2))  # (d, p, l)
    p2e = np.einsum("pdl,pdl->pl", patts, patts) + EPS
    for h in range(2):
        for bl in range(4):
            r = 12 * h + 3 * bl
            lhb[h, r : r + 3, 32 * bl : 32 * (bl + 1), :] = pdl
            lhb[h, 24 + 4 * h + bl, 32 * bl : 32 * (bl + 1), :] = 1.0
            lhb[h, 32, 32 * bl : 32 * (bl + 1), :] = p2e
    return lhb


def _build():
    if "nc" in _CACHE:
        return _CACHE["nc"]

    from contextlib import ExitStack

    import concourse.bass as bass  # noqa: F401
    import concourse.tile as tile
    from concourse import bacc, mybir

    f32 = mybir.dt.float32
    f32r = mybir.dt.float32r
    bf16 = mybir.dt.bfloat16
    AOT = mybir.AluOpType

    nc = bacc.Bacc(None, target_bir_lowering=False)
    x8 = nc.dram_tensor("x8", [BPC, ND, NT], f32, kind="ExternalInput")
    lhbase_d = nc.dram_tensor("lhbase", [2, K, 128, NL], f32r, kind="ExternalInput")
    w2inv24_d = nc.dram_tensor("w2inv24", [24, NT], f32, kind="ExternalInput")
    w2invp8_d = nc.dram_tensor("w2invp8", [8, NT], f32, kind="ExternalInput")
    w2inv1_d = nc.dram_tensor("w2inv1", [1, NT], f32r, kind="ExternalInput")
    einf1_d = nc.dram_tensor("einf1", [128, 1], bf16, kind="ExternalInput")
    wpos_d = nc.dram_tensor("wpos2", [128, 2, NT], f32, kind="ExternalInput")
    out_d = nc.dram_tensor("out8", [BPC, NP, NT], f32, kind="ExternalOutput")

    with tile.TileContext(nc) as tc:
        with ExitStack() as ctx:
            persist = ctx.enter_context(tc.tile_pool(name="persist", bufs=1))
            c_pool = ctx.enter_context(tc.tile_pool(name="cmin", bufs=2))
            psum_pool = ctx.enter_context(
                tc.tile_pool(name="psum", bufs=2, space="PSUM")
            )

            # lhsT per group: free layout (m, l), l contiguous for patts DMA
            lh = [persist.tile([K, 128, NL], f32r, name=f"lh{h}") for h in range(2)]
            xw = persist.tile([K, NT], f32r, name="xw")
            w2inv24 = persist.tile([24, NT], f32, name="w2inv24")
            w2invp8 = persist.tile([8, NT], f32, name="w2invp8")
            xg = persist.tile([24, NT], f32, name="xg")
            xa8 = persist.tile([8, ND, NT], f32, name="xa8")
            x2sum = persist.tile([8, NT], f32, name="x2sum")
            x2w = persist.tile([8, NT], f32r, name="x2w")
            wpos = persist.tile([128, 2, NT], f32, name="wpos")
            inf2 = persist.tile([128, NT // 2], f32, name="inf2")
            einf = persist.tile([128, 1], bf16, name="einf")
            E0 = persist.tile([128, NT2 + 1], bf16, name="E0")
            E1 = persist.tile([128, NT2 + 1], bf16, name="E1")
            E = [E0, E1]
            d3r = [persist.tile([128, NT2], f32, name=f"d3_{i}") for i in range(3)]
            oth = persist.tile([128, 2, NT], f32, name="oth")

            # ---------------- startup ----------------
            # critical path to the first scan: xa8 DMA -> squares -> x2sum
            # adds -> x2w -> xw[24:32] DMA -> matmuls -> sqrts.  Everything
            # else (p2e chain, lhbase, tables) is ordered to stay off it.
            actd = persist.tile([1, 1], f32, name="actd")
            nc.vector.memset(actd[:], 1.0)
            nc.scalar.sqrt(actd[:], actd[:])  # preload the Sqrt ACT table

            HT = NT // 2
            # x DMAs split by t-half so the x2 pipeline starts on half 0
            nc.scalar.dma_start(xa8[:, :, 0:HT], x8[:, :, 0:HT])
            nc.scalar.dma_start(xa8[:, :, HT:NT], x8[:, :, HT:NT])
            nc.sync.dma_start(w2inv24[:], w2inv24_d[:])
            nc.sync.dma_start(w2invp8[:], w2invp8_d[:])
            nc.sync.dma_start(xw[32:33, :], w2inv1_d[:])

            nc.gpsimd.dma_start(einf[:], einf1_d[:])
            nc.gpsimd.dma_start(xg[:], x8.rearrange("b d t -> (b d) t"))
            nc.gpsimd.dma_start(lh[0][:, :, :], lhbase_d[0])
            nc.gpsimd.dma_start(lh[1][:, :, :], lhbase_d[1])

            # rhs (xw) build, pipelined by t-half:
            # square -> two adds -> w2inv mult -> DMA into xw rows 24:32
            nc.vector.tensor_tensor(xw[0:24, :], xg[:], w2inv24[:], op=AOT.mult)
            for s in range(2):
                ts0, ts1 = s * HT, (s + 1) * HT
                nc.scalar.square(xa8[:, :, ts0:ts1], xa8[:, :, ts0:ts1])
                nc.vector.tensor_tensor(
                    x2sum[:, ts0:ts1], xa8[:, 0, ts0:ts1], xa8[:, 1, ts0:ts1],
                    op=AOT.add,
                )
                nc.vector.tensor_tensor(
                    x2sum[:, ts0:ts1], x2sum[:, ts0:ts1], xa8[:, 2, ts0:ts1],
                    op=AOT.add,
                )
                nc.vector.tensor_tensor(
                    x2w[:, ts0:ts1], x2sum[:, ts0:ts1], w2invp8[:, ts0:ts1],
                    op=AOT.mult,
                )
                nc.scalar.dma_start(xw[24:32, ts0:ts1], x2w[:, ts0:ts1])

            # ---------------- DP state init (DVE fills its idle window) ----
            nc.vector.memset(inf2[:], INF)
            # E1 edge slots; E0's edges are only read via the j==0 stash
            nc.vector.tensor_copy(out=E1[:, 0:1], in_=einf[:])
            nc.vector.tensor_copy(out=E1[:, NT + 1 : NT + 2], in_=einf[:])
            for i in range(3):
                nc.vector.memset(d3r[i][:, NT : NT + 1], LARGE)  # chain reset

            # ---------------- main loop over DP rows ----------------
            for j in range(NL):
                ps = psum_pool.tile([128, 2, NT], f32, name="ps")
                d3 = d3r[j % 3]
                for h in range(2):
                    nc.tensor.matmul(
                        ps[:, h, 0:512], lh[h][:, :, j], xw[:, 0:512],
                        start=True, stop=True,
                    )
                    nc.tensor.matmul(
                        ps[:, h, 512:1024], lh[h][:, :, j], xw[:, 512:1024],
                        start=True, stop=True,
                    )
                if j == 0:
                    # quarter sqrts so the first cumsum chunk starts as soon
                    # as the first matmul lands
                    for h in range(2):
                        k0 = 0 if h == 0 else NT + 1
                        nc.scalar.sqrt(d3[:, k0 : k0 + HT], ps[:, h, 0:HT])
                        nc.scalar.sqrt(
                            d3[:, k0 + HT : k0 + NT], ps[:, h, HT:NT]
                        )
                else:
                    nc.scalar.sqrt(d3[:, 0:NT], ps[:, 0, :])
                    nc.scalar.sqrt(d3[:, NT + 1 : NT2], ps[:, 1, :])

                Ecur, Eprev = E[j % 2], E[(j + 1) % 2]
                if j == 0:
                    # chunked cumsum chained via `initial`, following the
                    # quarter sqrts down the pipeline
                    for h in range(2):
                        for q in range(2):
                            lo = (1 if h == 0 else NT + 2) + q * HT
                            k0 = (0 if h == 0 else NT + 1) + q * HT
                            nc.vector.tensor_tensor_scan(
                                out=Ecur[:, lo : lo + HT],
                                data0=inf2[:],
                                data1=d3[:, k0 : k0 + HT],
                                initial=0.0 if q == 0 else Ecur[:, lo - 1 : lo],
                                op0=AOT.min,
                                op1=AOT.add,
                            )
                    # stash E0[t=0] per group into the edge slots so row 1's
                    # shifted-data0 view (cumsum monotonicity shortcut) is
                    # exact at t=0
                    nc.vector.tensor_copy(out=Ecur[:, 0:1], in_=Ecur[:, 1:2])
                    nc.vector.tensor_copy(
                        out=Ecur[:, NT + 1 : NT + 2], in_=Ecur[:, NT + 2 : NT + 3]
                    )
                elif j == 1:
                    # min(E0[t], E0[t-1]) == E0[t-1] by monotonicity; split
                    # per group to chase the sqrts
                    for h in range(2):
                        lo = 1 if h == 0 else NT + 2
                        k0 = 0 if h == 0 else NT + 1
                        nc.vector.tensor_tensor_scan(
                            out=Ecur[:, lo : lo + NT],
                            data0=Eprev[:, k0 : k0 + NT],
                            data1=d3[:, k0 : k0 + NT],
                            initial=INF,
                            op0=AOT.min,
                            op1=AOT.add,
                        )
                    # restore the INF edges for row 2's buffer reuse
                    nc.vector.tensor_copy(out=Eprev[:, 0:1], in_=einf[:])
                    nc.vector.tensor_copy(
                        out=Eprev[:, NT + 1 : NT + 2], in_=einf[:]
                    )
                elif j < NL - 1:
                    c3 = c_pool.tile([128, NT2], bf16, name="c3")
                    nc.vector.tensor_tensor(
                        c3[:], Eprev[:, 1 : NT2 + 1], Eprev[:, 0:NT2], op=AOT.min
                    )
                    nc.vector.tensor_tensor_scan(
                        out=Ecur[:, 1 : NT2 + 1],
                        data0=c3[:],
                        data1=d3[:, 0:NT2],
                        initial=INF,
                        op0=AOT.min,
                        op1=AOT.add,
                    )
                else:
                    # last row in quarter chunks: each chunk's rescale +
                    # store overlaps the remaining chunks' scans
                    of = out_d.rearrange("b p t -> (b p) t")
                    engs = [nc.sync, nc.scalar, nc.gpsimd, nc.sync]
                    c3 = c_pool.tile([128, NT2], bf16, name="c3")
                    nc.vector.tensor_tensor(
                        c3[:], Eprev[:, 1 : NT2 + 1], Eprev[:, 0:NT2], op=AOT.min
                    )
                    for h in range(2):
                        rows = slice(h * 128, (h + 1) * 128)
                        for q in range(2):
                            lo = (1 if h == 0 else NT + 2) + q * HT
                            k0 = (0 if h == 0 else NT + 1) + q * HT
                            t0 = q * HT
                            nc.vector.tensor_tensor_scan(
                                out=Ecur[:, lo : lo + HT],
                                data0=c3[:, k0 : k0 + HT],
                                data1=d3[:, k0 : k0 + HT],
                                initial=INF if q == 0 else Ecur[:, lo - 1 : lo],
                                op0=AOT.min,
                                op1=AOT.add,
                            )
                            nc.vector.tensor_tensor(
                                oth[:, h, t0 : t0 + HT],
                                Ecur[:, lo : lo + HT],
                                wpos[:, h, t0 : t0 + HT],
                                op=AOT.mult,
                            )
                            engs[2 * h + q].dma_start(
                                of[rows, t0 : t0 + HT], oth[:, h, t0 : t0 + HT]
                            )
                if j == 0:
                    # 1MB table only needed by the output stage; load it
                    # behind the startup DMAs
                    nc.gpsimd.dma_start(wpos[:], wpos_d[:])

    nc.compile()
    _CACHE["nc"] = nc
    return nc


def _in_maps(x, patts):
    W2INV24, W2INVP8, W2INV1, WPOS2, EINF1 = _tables()
    x = np.ascontiguousarray(np.asarray(x, dtype=np.float32))
    patts = np.ascontiguousarray(np.asarray(patts, dtype=np.float32))
    lhb = _lhbase(patts)
    maps = []
    for c in range(NCORES):
        maps.append(
            {
                "x8": np.ascontiguousarray(x[c * BPC : (c + 1) * BPC]),
                "lhbase": lhb,
                "w2inv24": W2INV24,
                "w2invp8": W2INVP8,
                "w2inv1": W2INV1,
                "einf1": EINF1,
                "wpos2": WPOS2,
            }
        )
    return maps


def kernel(x, patts):
    nc = _build()
    from concourse.bass_utils import run_bass_kernel_spmd

    res = run_bass_kernel_spmd(
        nc, _in_maps(x, patts), core_ids=list(range(NCORES))
    )
    _CACHE["last_results"] = res
    out = np.concatenate([r["out8"] for r in res.results], axis=0)
    return out.astype(np.float32)


# revision 31
# speedup vs baseline: 1.2847x; 1.0225x over previous
"""Trainium2 Bass kernel for DTWFeatures.

Problem: x (64,3,1024), patts (32,3,32) -> out (64,32,1024)
  dist[b,p,l,t] = sqrt(max(|x[b,:,t]-patts[p,:,l]|^2, eps))
  DP:  D[l,t] = dist[l,t] + min(D[l-1,t], w*D[l,t-1], w*D[l-1,t-1])
  out[b,p,t] = D[L-1,t]

Strategy (8 cores, data-parallel over batch, 8 batches/core, 256 (b,p)
pairs/core = 2 groups of 128 partitions):
  * Rescale E[l,t] = D[l,t]*w^-(t-SHIFT): removes w from the recurrence.
  * Per DP row the recurrence is one DVE tensor_tensor_scan
    (op0=min, op1=add) with data0 = windowed min of the previous row.
  * Both 128-pair groups are processed by ONE 2049-wide scan per row: a
    boundary element with data1=1e30 blows the scan state up to ~1e30,
    which the next element's min() discards -- an exact chain reset.
  * E rows are stored bf16 (scan state stays fp32 internally), so the
    window-min TensorTensor runs in the DVE 2x_1p mode (2 elem/cycle).
    dist stays fp32.  Validated: L2 rel err ~3e-3 (tolerance 2e-2).
  * dist'^2 comes from a single K=33 fp32r matmul per 512-col chunk
    (fp32r = 1 cycle/row vs fp32's 4): rhs rows = x*-2w2inv (24),
    x2*w2inv (8), w2inv (1); lhsT = block-diag patts / batch indicators /
    p2+eps, one lhsT per group.  EPS=1.5e-2 floors d^2 against fp32r
    cancellation noise (HW sqrt(neg)=NaN).
  * ScalarE applies sqrt PSUM->SBUF; DVE is the bottleneck engine at
    ~105us busy (32 scans of 2049 + 30 bf16 window-mins).
"""

import os
import sys

if "/opt/trn_rl_repo" not in sys.path:
    sys.path.insert(0, "/opt/trn_rl_repo")
# the device path runs through jax's axon PJRT backend; make sure a
# harness-pinned JAX_PLATFORMS doesn't hide it (no-op if jax is already up)
if "jax" not in sys.modules and "axon" not in os.environ.get(
    "JAX_PLATFORMS", "axon"
):
    os.environ["JAX_PLATFORMS"] = "axon," + os.environ["JAX_PLATFORMS"]

import numpy as np

NB, ND, NP, NL, NT = 64, 3, 32, 32, 1024   # batch, xdim, n_patts, l_patts, T
NCORES = 8
BPC = NB // NCORES                     # 8 batches per core
RHO = 0.1
W = RHO ** (1.0 / NL)
SHIFT = 512.0
EPS = 1.5e-2                           # floors d^2 against fp32r noise
INF = 1.0e30
LARGE = 1.0e30                         # scan chain-reset boundary value
K = 33                                 # matmul contraction rows
NT2 = 2 * NT + 1                       # merged scan length (1024+1+1024)

_CACHE = {}


def _tables():
    if "tables" not in _CACHE:
        import ml_dtypes

        t = np.arange(NT, dtype=np.float64)
        w2inv = (W ** (-2.0 * (t - SHIFT))).astype(np.float32)
        wpos = (W ** (t - SHIFT)).astype(np.float32)
        W2INV24 = np.ascontiguousarray(np.tile(w2inv[None, :], (24, 1)) * -2.0)
        W2INVP8 = np.ascontiguousarray(np.tile(w2inv[None, :], (8, 1)))
        W2INV1 = np.ascontiguousarray(w2inv[None, :])
        WPOS2 = np.ascontiguousarray(np.tile(wpos[None, None, :], (128, 2, 1)))
        EINF1 = np.full((128, 1), INF, np.float32).astype(ml_dtypes.bfloat16)
        _CACHE["tables"] = (W2INV24, W2INVP8, W2INV1, WPOS2, EINF1)
    return _CACHE["tables"]


def _lhbase(patts):
    """Full stationary lhsT per group: block-diagonal raw patts (rows 0:24,
    the -2 scale lives in the rhs), batch indicators (rows 24:32), and the
    p^2+eps row (row 32, paired with the rhs w2inv row)."""
    lhb = np.zeros((2, K, 128, NL), np.float32)
    pdl = np.transpose(patts, (1, 0, 2))  # (d, p, l)
    p2e = np.einsum("pdl,pdl->pl", patts, patts) + EPS
    for h in range(2):
        for bl in range(4):
            r = 12 * h + 3 * bl
            lhb[h, r : r + 3, 32 * bl : 32 * (bl + 1), :] = pdl
            lhb[h, 24 + 4 * h + bl, 32 * bl : 32 * (bl + 1), :] = 1.0
            lhb[h, 32, 32 * bl : 32 * (bl + 1), :] = p2e
    return lhb


def _build():
    if "nc" in _CACHE:
        return _CACHE["nc"]

    from contextlib import ExitStack

    import concourse.bass as bass  # noqa: F401
    import concourse.tile as tile
    from concourse import bacc, mybir

    f32 = mybir.dt.float32
    f32r = mybir.dt.float32r
    bf16 = mybir.dt.bfloat16
    AOT = mybir.AluOpType

    nc = bacc.Bacc(None, target_bir_lowering=False)
    x8 = nc.dram_tensor("x8", [BPC, ND, NT], f32, kind="ExternalInput")
    lhbase_d = nc.dram_tensor("lhbase", [2, K, 128, NL], f32r, kind="ExternalInput")
    w2inv24_d = nc.dram_tensor("w2inv24", [24, NT], f32, kind="ExternalInput")
    w2invp8_d = nc.dram_tensor("w2invp8", [8, NT], f32, kind="ExternalInput")
    w2inv1_d = nc.dram_tensor("w2inv1", [1, NT], f32r, kind="ExternalInput")
    einf1_d = nc.dram_tensor("einf1", [128, 1], bf16, kind="ExternalInput")
    wpos_d = nc.dram_tensor("wpos2", [128, 2, NT], f32, kind="ExternalInput")
    out_d = nc.dram_tensor("out8", [BPC, NP, NT], f32, kind="ExternalOutput")

    with tile.TileContext(nc) as tc:
        with ExitStack() as ctx:
            persist = ctx.enter_context(tc.tile_pool(name="persist", bufs=1))
            c_pool = ctx.enter_context(tc.tile_pool(name="cmin", bufs=2))
            psum_pool = ctx.enter_context(
                tc.tile_pool(name="psum", bufs=1, space="PSUM")
            )
            # j=0 runs through four single-matmul PSUM quarter tiles so each
            # sqrt chunk only waits on its own matmul
            q_pool = ctx.enter_context(
                tc.tile_pool(name="psumq", bufs=4, space="PSUM")
            )

            HT = NT // 2
            # lhsT per group: free layout (m, l), l contiguous for patts DMA
            lh = [persist.tile([K, 128, NL], f32r, name=f"lh{h}") for h in range(2)]
            # rhs split by column half so each matmul waits only on its half
            xwh = [persist.tile([K, HT], f32r, name=f"xw{s}") for s in range(2)]
            w2inv24h = [persist.tile([24, HT], f32, name=f"w2inv24{s}") for s in range(2)]
            w2invp8 = persist.tile([8, NT], f32, name="w2invp8")
            xgh = [persist.tile([24, HT], f32, name=f"xg{s}") for s in range(2)]
            xa8 = persist.tile([8, ND, NT], f32, name="xa8")
            x2sum = persist.tile([8, NT], f32, name="x2sum")
            x2w = persist.tile([8, NT], f32r, name="x2w")
            wpos = persist.tile([128, 2, NT], f32, name="wpos")
            inf2 = persist.tile([128, NT // 2], f32, name="inf2")
            einf = persist.tile([128, 1], bf16, name="einf")
            E0 = persist.tile([128, NT2 + 1], bf16, name="E0")
            E1 = persist.tile([128, NT2 + 1], bf16, name="E1")
            E = [E0, E1]
            d3r = [persist.tile([128, NT2], f32, name=f"d3_{i}") for i in range(3)]
            oth = persist.tile([128, 2, NT], f32, name="oth")

            # ---------------- startup ----------------
            # critical path to the first scan: xa8 DMA -> squares -> x2sum
            # adds -> x2w -> xw[24:32] DMA -> matmuls -> sqrts.  Everything
            # else (p2e chain, lhbase, tables) is ordered to stay off it.
            actd = persist.tile([1, 1], f32, name="actd")
            nc.vector.memset(actd[:], 1.0)
            nc.scalar.sqrt(actd[:], actd[:])  # preload the Sqrt ACT table

            xgf = x8.rearrange("b d t -> (b d) t")
            # x DMAs split by t-half so the x2 pipeline starts on half 0;
            # sync-queue order tracks the compute order (half 0 before
            # lhbase before half 1)
            nc.scalar.dma_start(xa8[:, :, 0:HT], x8[:, :, 0:HT])
            nc.scalar.dma_start(xa8[:, :, HT:NT], x8[:, :, HT:NT])
            nc.sync.dma_start(xgh[0][:], xgf[:, 0:HT])
            nc.sync.dma_start(w2inv24h[0][:], w2inv24_d[:, 0:HT])
            nc.sync.dma_start(w2invp8[:], w2invp8_d[:])
            nc.sync.dma_start(xwh[0][32:33, :], w2inv1_d[:, 0:HT])
            nc.sync.dma_start(xwh[1][32:33, :], w2inv1_d[:, HT:NT])
            nc.sync.dma_start(lh[0][:, :, :], lhbase_d[0])
            nc.sync.dma_start(xgh[1][:], xgf[:, HT:NT])
            nc.sync.dma_start(w2inv24h[1][:], w2inv24_d[:, HT:NT])
            nc.sync.dma_start(lh[1][:, :, :], lhbase_d[1])
            nc.gpsimd.dma_start(einf[:], einf1_d[:])

            # rhs (xw) build, pipelined by t-half:
            # square -> two adds -> w2inv mult -> DMA into xw rows 24:32
            for s in range(2):
                ts0, ts1 = s * HT, (s + 1) * HT
                nc.scalar.square(xa8[:, :, ts0:ts1], xa8[:, :, ts0:ts1])
                nc.vector.tensor_tensor(
                    xwh[s][0:24, :], xgh[s][:], w2inv24h[s][:], op=AOT.mult
                )
                nc.vector.tensor_tensor(
                    x2sum[:, ts0:ts1], xa8[:, 0, ts0:ts1], xa8[:, 1, ts0:ts1],
                    op=AOT.add,
                )
                nc.vector.tensor_tensor(
                    x2sum[:, ts0:ts1], x2sum[:, ts0:ts1], xa8[:, 2, ts0:ts1],
                    op=AOT.add,
                )
                nc.vector.tensor_tensor(
                    x2w[:, ts0:ts1], x2sum[:, ts0:ts1], w2invp8[:, ts0:ts1],
                    op=AOT.mult,
                )
                nc.scalar.dma_start(xwh[s][24:32, :], x2w[:, ts0:ts1])

            # ---------------- DP state init (DVE fills its idle window) ----
            nc.vector.memset(inf2[:], INF)
            # E1 edge slots; E0's edges are only read via the j==0 stash
            nc.vector.tensor_copy(out=E1[:, 0:1], in_=einf[:])
            nc.vector.tensor_copy(out=E1[:, NT + 1 : NT + 2], in_=einf[:])
            for i in range(3):
                nc.vector.memset(d3r[i][:, NT : NT + 1], LARGE)  # chain reset

            # ---------------- main loop over DP rows ----------------
            for j in range(NL):
                d3 = d3r[j % 3]
                if j == 0:
                    # four independent matmul->sqrt quarter pipelines;
                    # column-half 0 (ready first) for both groups, then half 1
                    for q in range(2):
                        for h in range(2):
                            k0 = 0 if h == 0 else NT + 1
                            psq = q_pool.tile([128, HT], f32, name="psq")
                            nc.tensor.matmul(
                                psq[:, :], lh[h][:, :, j], xwh[q][:, :],
                                start=True, stop=True,
                            )
                            nc.scalar.sqrt(
                                d3[:, k0 + q * HT : k0 + (q + 1) * HT], psq[:, :]
                            )
                else:
                    ps = psum_pool.tile([128, 2, NT], f32, name="ps")
                    for h in range(2):
                        nc.tensor.matmul(
                            ps[:, h, 0:HT], lh[h][:, :, j], xwh[0][:, :],
                            start=True, stop=True,
                        )
                        nc.tensor.matmul(
                            ps[:, h, HT:NT], lh[h][:, :, j], xwh[1][:, :],
                            start=True, stop=True,
                        )
                    nc.scalar.sqrt(d3[:, 0:NT], ps[:, 0, :])
                    nc.scalar.sqrt(d3[:, NT + 1 : NT2], ps[:, 1, :])

                Ecur, Eprev = E[j % 2], E[(j + 1) % 2]
                if j == 0:
                    # chunked cumsum chained via `initial`, following the
                    # quarter sqrts down the pipeline
                    for q in range(2):
                        for h in range(2):
                            lo = (1 if h == 0 else NT + 2) + q * HT
                            k0 = (0 if h == 0 else NT + 1) + q * HT
                            nc.vector.tensor_tensor_scan(
                                out=Ecur[:, lo : lo + HT],
                                data0=inf2[:],
                                data1=d3[:, k0 : k0 + HT],
                                initial=0.0 if q == 0 else Ecur[:, lo - 1 : lo],
                                op0=AOT.min,
                                op1=AOT.add,
                            )
                    # stash E0[t=0] per group into the edge slots so row 1's
                    # shifted-data0 view (cumsum monotonicity shortcut) is
                    # exact at t=0
                    nc.vector.tensor_copy(out=Ecur[:, 0:1], in_=Ecur[:, 1:2])
                    nc.vector.tensor_copy(
                        out=Ecur[:, NT + 1 : NT + 2], in_=Ecur[:, NT + 2 : NT + 3]
                    )
                elif j == 1:
                    # min(E0[t], E0[t-1]) == E0[t-1] by monotonicity; split
                    # per group to chase the sqrts
                    for h in range(2):
                        lo = 1 if h == 0 else NT + 2
                        k0 = 0 if h == 0 else NT + 1
                        nc.vector.tensor_tensor_scan(
                            out=Ecur[:, lo : lo + NT],
                            data0=Eprev[:, k0 : k0 + NT],
                            data1=d3[:, k0 : k0 + NT],
                            initial=INF,
                            op0=AOT.min,
                            op1=AOT.add,
                        )
                    # restore the INF edges for row 2's buffer reuse
                    nc.vector.tensor_copy(out=Eprev[:, 0:1], in_=einf[:])
                    nc.vector.tensor_copy(
                        out=Eprev[:, NT + 1 : NT + 2], in_=einf[:]
                    )
                elif j < NL - 1:
                    c3 = c_pool.tile([128, NT2], bf16, name="c3")
                    nc.vector.tensor_tensor(
                        c3[:], Eprev[:, 1 : NT2 + 1], Eprev[:, 0:NT2], op=AOT.min
                    )
                    nc.vector.tensor_tensor_scan(
                        out=Ecur[:, 1 : NT2 + 1],
                        data0=c3[:],
                        data1=d3[:, 0:NT2],
                        initial=INF,
                        op0=AOT.min,
                        op1=AOT.add,
                    )
                else:
                    # last row in quarter chunks: each chunk's rescale +
                    # store overlaps the remaining chunks' scans
                    of = out_d.rearrange("b p t -> (b p) t")
                    engs = [nc.sync, nc.scalar, nc.gpsimd, nc.sync]
                    c3 = c_pool.tile([128, NT2], bf16, name="c3")
                    nc.vector.tensor_tensor(
                        c3[:], Eprev[:, 1 : NT2 + 1], Eprev[:, 0:NT2], op=AOT.min
                    )
                    for h in range(2):
                        rows = slice(h * 128, (h + 1) * 128)
                        for q in range(2):
                            lo = (1 if h == 0 else NT + 2) + q * HT
                            k0 = (0 if h == 0 else NT + 1) + q * HT
                            t0 = q * HT
                            nc.vector.tensor_tensor_scan(
                                out=Ecur[:, lo : lo + HT],
                                data0=c3[:, k0 : k0 + HT],
                                data1=d3[:, k0 : k0 + HT],
                                initial=INF if q == 0 else Ecur[:, lo - 1 : lo],
                                op0=AOT.min,
                                op1=AOT.add,
                            )
                            nc.vector.tensor_tensor(
                                oth[:, h, t0 : t0 + HT],
                                Ecur[:, lo : lo + HT],
                                wpos[:, h, t0 : t0 + HT],
                                op=AOT.mult,
                            )
                            engs[2 * h + q].dma_start(
                                of[rows, t0 : t0 + HT], oth[:, h, t0 : t0 + HT]
                            )
                if j == 0:
                    # 1MB table only needed by the output stage; load it
                    # behind the startup DMAs
                    nc.gpsimd.dma_start(wpos[:], wpos_d[:])

    nc.compile()
    _CACHE["nc"] = nc
    return nc


def _in_maps(x, patts):
    W2INV24, W2INVP8, W2INV1, WPOS2, EINF1 = _tables()
    x = np.ascontiguousarray(np.asarray(x, dtype=np.float32))
    patts = np.ascontiguousarray(np.asarray(patts, dtype=np.float32))
    lhb = _lhbase(patts)
    maps = []
    for c in range(NCORES):
        maps.append(
            {
                "x8": np.ascontiguousarray(x[c * BPC : (c + 1) * BPC]),
                "lhbase": lhb,
                "w2inv24": W2INV24,
                "w2invp8": W2INVP8,
                "w2inv1": W2INV1,
                "einf1": EINF1,
                "wpos2": WPOS2,
            }
        )
    return maps


def kernel(x, patts):
    nc = _build()
    from concourse.bass_utils import run_bass_kernel_spmd

    res = run_bass_kernel_spmd(
        nc, _in_maps(x, patts), core_ids=list(range(NCORES))
    )
    _CACHE["last_results"] = res
    out = np.concatenate([r["out8"] for r in res.results], axis=0)
    return out.astype(np.float32)


# revision 33
# speedup vs baseline: 1.3040x; 1.0150x over previous
"""Trainium2 Bass kernel for DTWFeatures.

Problem: x (64,3,1024), patts (32,3,32) -> out (64,32,1024)
  dist[b,p,l,t] = sqrt(max(|x[b,:,t]-patts[p,:,l]|^2, eps))
  DP:  D[l,t] = dist[l,t] + min(D[l-1,t], w*D[l,t-1], w*D[l-1,t-1])
  out[b,p,t] = D[L-1,t]

Strategy (8 cores, data-parallel over batch, 8 batches/core, 256 (b,p)
pairs/core = 2 groups of 128 partitions):
  * Rescale E[l,t] = D[l,t]*w^-(t-SHIFT): removes w from the recurrence.
  * Per DP row the recurrence is one DVE tensor_tensor_scan
    (op0=min, op1=add) with data0 = windowed min of the previous row.
  * Both 128-pair groups are processed by ONE 2049-wide scan per row: a
    boundary element with data1=1e30 blows the scan state up to ~1e30,
    which the next element's min() discards -- an exact chain reset.
  * E rows are stored bf16 (scan state stays fp32 internally), so the
    window-min TensorTensor runs in the DVE 2x_1p mode (2 elem/cycle).
    dist stays fp32.  Validated: L2 rel err ~3e-3 (tolerance 2e-2).
  * dist'^2 comes from a single K=33 fp32r matmul per 512-col chunk
    (fp32r = 1 cycle/row vs fp32's 4): rhs rows = x*-2w2inv (24),
    x2*w2inv (8), w2inv (1); lhsT = block-diag patts / batch indicators /
    p2+eps, one lhsT per group.  EPS=1.5e-2 floors d^2 against fp32r
    cancellation noise (HW sqrt(neg)=NaN).
  * ScalarE applies sqrt PSUM->SBUF; DVE is the bottleneck engine at
    ~105us busy (32 scans of 2049 + 30 bf16 window-mins).
"""

import os
import sys

if "/opt/trn_rl_repo" not in sys.path:
    sys.path.insert(0, "/opt/trn_rl_repo")
# the device path runs through jax's axon PJRT backend; make sure a
# harness-pinned JAX_PLATFORMS doesn't hide it (no-op if jax is already up)
if "jax" not in sys.modules and "axon" not in os.environ.get(
    "JAX_PLATFORMS", "axon"
):
    os.environ["JAX_PLATFORMS"] = "axon," + os.environ["JAX_PLATFORMS"]

import numpy as np

NB, ND, NP, NL, NT = 64, 3, 32, 32, 1024   # batch, xdim, n_patts, l_patts, T
NCORES = 8
BPC = NB // NCORES                     # 8 batches per core
RHO = 0.1
W = RHO ** (1.0 / NL)
SHIFT = 512.0
EPS = 1.5e-2                           # floors d^2 against fp32r noise
INF = 1.0e30
LARGE = 1.0e30                         # scan chain-reset boundary value
K = 33                                 # matmul contraction rows
NT2 = 2 * NT + 1                       # merged scan length (1024+1+1024)

_CACHE = {}


def _tables():
    if "tables" not in _CACHE:
        import ml_dtypes

        t = np.arange(NT, dtype=np.float64)
        w2inv = (W ** (-2.0 * (t - SHIFT))).astype(np.float32)
        wpos = (W ** (t - SHIFT)).astype(np.float32)
        W2INV24 = np.ascontiguousarray(np.tile(w2inv[None, :], (24, 1)) * -2.0)
        W2INVP8 = np.ascontiguousarray(np.tile(w2inv[None, :], (8, 1)))
        W2INV1 = np.ascontiguousarray(w2inv[None, :])
        WPOS2 = np.ascontiguousarray(np.tile(wpos[None, None, :], (128, 2, 1)))
        EINF1 = np.full((128, 1), INF, np.float32).astype(ml_dtypes.bfloat16)
        _CACHE["tables"] = (W2INV24, W2INVP8, W2INV1, WPOS2, EINF1)
    return _CACHE["tables"]


def _lhbase(patts):
    """Full stationary lhsT per group: block-diagonal raw patts (rows 0:24,
    the -2 scale lives in the rhs), batch indicators (rows 24:32), and the
    p^2+eps row (row 32, paired with the rhs w2inv row)."""
    lhb = np.zeros((2, K, 128, NL), np.float32)
    pdl = np.transpose(patts, (1, 0, 2))  # (d, p, l)
    p2e = np.einsum("pdl,pdl->pl", patts, patts) + EPS
    for h in range(2):
        for bl in range(4):
            r = 12 * h + 3 * bl
            lhb[h, r : r + 3, 32 * bl : 32 * (bl + 1), :] = pdl
            lhb[h, 24 + 4 * h + bl, 32 * bl : 32 * (bl + 1), :] = 1.0
            lhb[h, 32, 32 * bl : 32 * (bl + 1), :] = p2e
    return lhb


def _build():
    if "nc" in _CACHE:
        return _CACHE["nc"]

    from contextlib import ExitStack

    import concourse.bass as bass  # noqa: F401
    import concourse.tile as tile
    from concourse import bacc, mybir

    f32 = mybir.dt.float32
    f32r = mybir.dt.float32r
    bf16 = mybir.dt.bfloat16
    AOT = mybir.AluOpType

    nc = bacc.Bacc(None, target_bir_lowering=False)
    x8 = nc.dram_tensor("x8", [BPC, ND, NT], f32, kind="ExternalInput")
    lhbase_d = nc.dram_tensor("lhbase", [2, K, 128, NL], f32r, kind="ExternalInput")
    w2inv24_d = nc.dram_tensor("w2inv24", [24, NT], f32, kind="ExternalInput")
    w2invp8_d = nc.dram_tensor("w2invp8", [8, NT], f32, kind="ExternalInput")
    w2inv1_d = nc.dram_tensor("w2inv1", [1, NT], f32r, kind="ExternalInput")
    einf1_d = nc.dram_tensor("einf1", [128, 1], bf16, kind="ExternalInput")
    wpos_d = nc.dram_tensor("wpos2", [128, 2, NT], f32, kind="ExternalInput")
    out_d = nc.dram_tensor("out8", [BPC, NP, NT], f32, kind="ExternalOutput")

    with tile.TileContext(nc) as tc:
        with ExitStack() as ctx:
            persist = ctx.enter_context(tc.tile_pool(name="persist", bufs=1))
            c_pool = ctx.enter_context(tc.tile_pool(name="cmin", bufs=2))
            # [128, NT] half-row tiles: sqrt(j,h) only waits on group h's two
            # matmuls, and 4 bufs give two rows of pipeline slack
            psum_pool = ctx.enter_context(
                tc.tile_pool(name="psum", bufs=4, space="PSUM")
            )

            HT = NT // 2
            # lhsT per group: free layout (m, l), l contiguous for patts DMA
            lh = [persist.tile([K, 128, NL], f32r, name=f"lh{h}") for h in range(2)]
            # rhs split by column half so each matmul waits only on its half
            xwh = [persist.tile([K, HT], f32r, name=f"xw{s}") for s in range(2)]
            w2inv24h = [persist.tile([24, HT], f32, name=f"w2inv24{s}") for s in range(2)]
            w2invp8 = persist.tile([8, NT], f32, name="w2invp8")
            xgh = [persist.tile([24, HT], f32, name=f"xg{s}") for s in range(2)]
            xa8 = persist.tile([8, ND, NT], f32, name="xa8")
            x2sum = persist.tile([8, NT], f32, name="x2sum")
            x2w = persist.tile([8, NT], f32r, name="x2w")
            wpos = persist.tile([128, 2, NT], f32, name="wpos")
            inf2 = persist.tile([128, NT // 2], f32, name="inf2")
            einf = persist.tile([128, 1], bf16, name="einf")
            E0 = persist.tile([128, NT2 + 1], bf16, name="E0")
            E1 = persist.tile([128, NT2 + 1], bf16, name="E1")
            E = [E0, E1]
            d3r = [persist.tile([128, NT2], f32, name=f"d3_{i}") for i in range(3)]
            oth = persist.tile([128, 2, NT], f32, name="oth")

            # ---------------- startup ----------------
            # critical path to the first scan: xa8 DMA -> squares -> x2sum
            # adds -> x2w -> xw[24:32] DMA -> matmuls -> sqrts.  Everything
            # else (p2e chain, lhbase, tables) is ordered to stay off it.
            actd = persist.tile([1, 1], f32, name="actd")
            nc.vector.memset(actd[:], 1.0)
            nc.scalar.sqrt(actd[:], actd[:])  # preload the Sqrt ACT table

            xgf = x8.rearrange("b d t -> (b d) t")
            # x DMAs split by t-half so the x2 pipeline starts on half 0;
            # sync-queue order tracks the compute order (half 0 before
            # lhbase before half 1)
            nc.scalar.dma_start(xa8[:, :, 0:HT], x8[:, :, 0:HT])
            nc.scalar.dma_start(xa8[:, :, HT:NT], x8[:, :, HT:NT])
            nc.sync.dma_start(xgh[0][:], xgf[:, 0:HT])
            nc.sync.dma_start(w2inv24h[0][:], w2inv24_d[:, 0:HT])
            nc.sync.dma_start(w2invp8[:], w2invp8_d[:])
            nc.sync.dma_start(xwh[0][32:33, :], w2inv1_d[:, 0:HT])
            nc.sync.dma_start(xwh[1][32:33, :], w2inv1_d[:, HT:NT])
            nc.sync.dma_start(lh[0][:, :, :], lhbase_d[0])
            nc.sync.dma_start(xgh[1][:], xgf[:, HT:NT])
            nc.sync.dma_start(w2inv24h[1][:], w2inv24_d[:, HT:NT])
            nc.sync.dma_start(lh[1][:, :, :], lhbase_d[1])
            nc.gpsimd.dma_start(einf[:], einf1_d[:])

            # rhs (xw) build, pipelined by t-half:
            # square -> two adds -> w2inv mult -> DMA into xw rows 24:32
            for s in range(2):
                ts0, ts1 = s * HT, (s + 1) * HT
                nc.scalar.square(xa8[:, :, ts0:ts1], xa8[:, :, ts0:ts1])
                nc.vector.tensor_tensor(
                    xwh[s][0:24, :], xgh[s][:], w2inv24h[s][:], op=AOT.mult
                )
                nc.vector.tensor_tensor(
                    x2sum[:, ts0:ts1], xa8[:, 0, ts0:ts1], xa8[:, 1, ts0:ts1],
                    op=AOT.add,
                )
                nc.vector.tensor_tensor(
                    x2sum[:, ts0:ts1], x2sum[:, ts0:ts1], xa8[:, 2, ts0:ts1],
                    op=AOT.add,
                )
                nc.vector.tensor_tensor(
                    x2w[:, ts0:ts1], x2sum[:, ts0:ts1], w2invp8[:, ts0:ts1],
                    op=AOT.mult,
                )
                nc.scalar.dma_start(xwh[s][24:32, :], x2w[:, ts0:ts1])

            # ---------------- DP state init (DVE fills its idle window) ----
            nc.vector.memset(inf2[:], INF)
            # E1 edge slots; E0's edges are only read via the j==0 stash
            nc.vector.tensor_copy(out=E1[:, 0:1], in_=einf[:])
            nc.vector.tensor_copy(out=E1[:, NT + 1 : NT + 2], in_=einf[:])
            for i in range(3):
                nc.vector.memset(d3r[i][:, NT : NT + 1], LARGE)  # chain reset

            # ---------------- main loop over DP rows ----------------
            for j in range(NL):
                d3 = d3r[j % 3]
                if j == 0:
                    # four independent matmul->sqrt quarter pipelines;
                    # column-half 0 (ready first) for both groups, then half 1
                    for q in range(2):
                        for h in range(2):
                            k0 = 0 if h == 0 else NT + 1
                            psq = psum_pool.tile([128, NT], f32, name="ps")
                            nc.tensor.matmul(
                                psq[:, 0:HT], lh[h][:, :, j], xwh[q][:, :],
                                start=True, stop=True,
                            )
                            nc.scalar.sqrt(
                                d3[:, k0 + q * HT : k0 + (q + 1) * HT],
                                psq[:, 0:HT],
                            )
                else:
                    for h in range(2):
                        k0 = 0 if h == 0 else NT + 1
                        ps = psum_pool.tile([128, NT], f32, name="ps")
                        nc.tensor.matmul(
                            ps[:, 0:HT], lh[h][:, :, j], xwh[0][:, :],
                            start=True, stop=True,
                        )
                        nc.tensor.matmul(
                            ps[:, HT:NT], lh[h][:, :, j], xwh[1][:, :],
                            start=True, stop=True,
                        )
                        nc.scalar.sqrt(d3[:, k0 : k0 + NT], ps[:, :])

                Ecur, Eprev = E[j % 2], E[(j + 1) % 2]
                if j == 0:
                    # chunked cumsum chained via `initial`, following the
                    # quarter sqrts down the pipeline
                    for q in range(2):
                        for h in range(2):
                            lo = (1 if h == 0 else NT + 2) + q * HT
                            k0 = (0 if h == 0 else NT + 1) + q * HT
                            nc.vector.tensor_tensor_scan(
                                out=Ecur[:, lo : lo + HT],
                                data0=inf2[:],
                                data1=d3[:, k0 : k0 + HT],
                                initial=0.0 if q == 0 else Ecur[:, lo - 1 : lo],
                                op0=AOT.min,
                                op1=AOT.add,
                            )
                    # stash E0[t=0] per group into the edge slots so row 1's
                    # shifted-data0 view (cumsum monotonicity shortcut) is
                    # exact at t=0
                    nc.vector.tensor_copy(out=Ecur[:, 0:1], in_=Ecur[:, 1:2])
                    nc.vector.tensor_copy(
                        out=Ecur[:, NT + 1 : NT + 2], in_=Ecur[:, NT + 2 : NT + 3]
                    )
                elif j == 1:
                    # min(E0[t], E0[t-1]) == E0[t-1] by monotonicity; split
                    # per group to chase the sqrts
                    for h in range(2):
                        lo = 1 if h == 0 else NT + 2
                        k0 = 0 if h == 0 else NT + 1
                        nc.vector.tensor_tensor_scan(
                            out=Ecur[:, lo : lo + NT],
                            data0=Eprev[:, k0 : k0 + NT],
                            data1=d3[:, k0 : k0 + NT],
                            initial=INF,
                            op0=AOT.min,
                            op1=AOT.add,
                        )
                    # restore the INF edges for row 2's buffer reuse
                    nc.vector.tensor_copy(out=Eprev[:, 0:1], in_=einf[:])
                    nc.vector.tensor_copy(
                        out=Eprev[:, NT + 1 : NT + 2], in_=einf[:]
                    )
                elif j < NL - 1:
                    c3 = c_pool.tile([128, NT2], bf16, name="c3")
                    nc.vector.tensor_tensor(
                        c3[:], Eprev[:, 1 : NT2 + 1], Eprev[:, 0:NT2], op=AOT.min
                    )
                    nc.vector.tensor_tensor_scan(
                        out=Ecur[:, 1 : NT2 + 1],
                        data0=c3[:],
                        data1=d3[:, 0:NT2],
                        initial=INF,
                        op0=AOT.min,
                        op1=AOT.add,
                    )
                else:
                    # last row in quarter chunks: each chunk's rescale +
                    # store overlaps the remaining chunks' scans
                    of = out_d.rearrange("b p t -> (b p) t")
                    engs = [nc.sync, nc.scalar, nc.gpsimd, nc.sync]
                    c3 = c_pool.tile([128, NT2], bf16, name="c3")
                    nc.vector.tensor_tensor(
                        c3[:], Eprev[:, 1 : NT2 + 1], Eprev[:, 0:NT2], op=AOT.min
                    )
                    for h in range(2):
                        rows = slice(h * 128, (h + 1) * 128)
                        for q in range(2):
                            lo = (1 if h == 0 else NT + 2) + q * HT
                            k0 = (0 if h == 0 else NT + 1) + q * HT
                            t0 = q * HT
                            nc.vector.tensor_tensor_scan(
                                out=Ecur[:, lo : lo + HT],
                                data0=c3[:, k0 : k0 + HT],
                                data1=d3[:, k0 : k0 + HT],
                                initial=INF if q == 0 else Ecur[:, lo - 1 : lo],
                                op0=AOT.min,
                                op1=AOT.add,
                            )
                            nc.vector.tensor_tensor(
                                oth[:, h, t0 : t0 + HT],
                                Ecur[:, lo : lo + HT],
                                wpos[:, h, t0 : t0 + HT],
                                op=AOT.mult,
                            )
                            engs[2 * h + q].dma_start(
                                of[rows, t0 : t0 + HT], oth[:, h, t0 : t0 + HT]
                            )
                if j == 0:
                    # 1MB table only needed by the output stage; load it
                    # behind the startup DMAs
                    nc.gpsimd.dma_start(wpos[:], wpos_d[:])

    nc.compile()
    _CACHE["nc"] = nc
    return nc


def _in_maps(x, patts):
    W2INV24, W2INVP8, W2INV1, WPOS2, EINF1 = _tables()
    x = np.ascontiguousarray(np.asarray(x, dtype=np.float32))
    patts = np.ascontiguousarray(np.asarray(patts, dtype=np.float32))
    lhb = _lhbase(patts)
    maps = []
    for c in range(NCORES):
        maps.append(
            {
                "x8": np.ascontiguousarray(x[c * BPC : (c + 1) * BPC]),
                "lhbase": lhb,
                "w2inv24": W2INV24,
                "w2invp8": W2INVP8,
                "w2inv1": W2INV1,
                "einf1": EINF1,
                "wpos2": WPOS2,
            }
        )
    return maps


def kernel(x, patts):
    nc = _build()
    from concourse.bass_utils import run_bass_kernel_spmd

    res = run_bass_kernel_spmd(
        nc, _in_maps(x, patts), core_ids=list(range(NCORES))
    )
    _CACHE["last_results"] = res
    out = np.concatenate([r["out8"] for r in res.results], axis=0)
    return out.astype(np.float32)
